# revision 8
# baseline (speedup 1.0000x reference)
"""AttnBlock kernel for 8 Trainium2 NeuronCores — single-pass fp16, v3.

Sharding: 8 cores = 4 examples x 2 query-halves (columns rotated per core so
its half comes first; softmax over keys is permutation invariant). No
cross-core communication.

v3 structure: x is DMA'd once (phase 1), stats via bn_stats, and an fp16 copy
of x is kept in SBUF for the convs and the residual (saves 16 MB of HBM
re-reads). Convs and scores are single-pass fp16 (validated rel_err ~1e-2 on
the fixed inputs). Attention uses a 2-half online softmax with chunk-contiguous
score matmuls (per-chunk maxes overlap the score stream), 5 rotating score
PSUM banks so pass B starts before all of pass A is consumed, interleaved
transpose/attnV groups, and the proj of each 4-block group spread across the
following iteration.
"""

import math

import numpy as np

import concourse.bacc as bacc
import concourse.mybir as mybir
import concourse.tile as tile
from concourse.bass_utils import run_bass_kernel_spmd
from concourse.masks import make_identity

F32 = mybir.dt.float32
F16 = mybir.dt.float16

B, C, H, W = 4, 512, 64, 64
N = H * W            # 4096 key positions
NQ = N // 2          # 2048 query positions per core
P = 128              # partitions
CT = C // P          # 4 channel tiles
NCH = N // 512       # 8 key chunks of 512
NQB = NQ // P        # 16 query blocks of 128
G = 2                # groupnorm groups
EPS = 1e-6
AX = mybir.AxisListType.X
ALU = mybir.AluOpType
ACTF = mybir.ActivationFunctionType

_CACHED_NC = None


def build_nc(loop_r: int = 1):
    nc = bacc.Bacc("TRN2", target_bir_lowering=False)

    x_d = nc.dram_tensor("x", [CT, P, N], F16, kind="ExternalInput")
    wqt_d = nc.dram_tensor("wqt", [P, CT, C], F16, kind="ExternalInput")  # [p, t, o], q scaled by sqrt(C)
    wkt_d = nc.dram_tensor("wkt", [P, CT, C], F16, kind="ExternalInput")
    wvt_d = nc.dram_tensor("wvt", [P, CT, C], F16, kind="ExternalInput")
    wpt_d = nc.dram_tensor("wpt", [P, CT, C], F16, kind="ExternalInput")
    # per-channel params packed: [p, t, (bq, bk, bp', gnw, gnb, pad)]
    # bp' = bp + wp @ bv  (v-bias folded into proj bias; attn rows sum to 1)
    prm_d = nc.dram_tensor("prm", [P, CT, 6], F32, kind="ExternalInput")
    out_d = nc.dram_tensor("out", [CT, P, NQ], F32, kind="ExternalOutput")

    import contextlib

    with tile.TileContext(nc) as tc:
        loop_ctx = tc.For_i(0, loop_r, 1) if loop_r > 1 else contextlib.nullcontext()
        with (
            loop_ctx,
            tc.tile_pool(name="singles", bufs=1) as singles,
            tc.tile_pool(name="persist", bufs=1) as persist,
            tc.tile_pool(name="h16_pool", bufs=8) as h16_pool,
        ):
            ident = singles.tile([P, P], F16, name="ident")
            make_identity(nc, ident)
            ones_f32 = singles.tile([P, P], F32, name="ones_f32")
            nc.vector.memset(ones_f32, 1.0)
            inv256 = singles.tile([P, 1], F32, name="inv256")
            nc.vector.memset(inv256, 1.0 / 256.0)
            eps_t = singles.tile([P, 1], F32, name="eps_t")
            nc.vector.memset(eps_t, EPS)
            inv4096 = singles.tile([P, 1], F32, name="inv4096")
            nc.vector.memset(inv4096, 1.0 / N)

            # weights and per-channel params: one packed DMA each
            wqt_all = persist.tile([P, CT, C], F16, name="wqt_all")
            wkt_all = persist.tile([P, CT, C], F16, name="wkt_all")
            wvt_all = persist.tile([P, CT, C], F16, name="wvt_all")
            wpt_all = persist.tile([P, CT, C], F16, name="wpt_all")
            prm = persist.tile([P, CT, 6], F32, name="prm")
            # prm loads early (needed for the stats tail); all weights queue on
            # the sync queue behind the phase-1 x stream so they don't steal
            # HBM bandwidth from it — they arrive just in time for phase 2
            nc.gpsimd.dma_start(out=prm, in_=prm_d[:, :, :])
            nc.gpsimd.dma_start(out=wkt_all, in_=wkt_d[:, :, :])
            nc.gpsimd.dma_start(out=wqt_all, in_=wqt_d[:, :, :])
            wqt = [wqt_all[:, t, :] for t in range(CT)]
            wkt = [wkt_all[:, t, :] for t in range(CT)]
            wvt = [wvt_all[:, t, :] for t in range(CT)]
            wpt = [wpt_all[:, t, :] for t in range(CT)]
            bq = [prm[:, t, 0:1] for t in range(CT)]
            bk = [prm[:, t, 1:2] for t in range(CT)]
            bp = [prm[:, t, 2:3] for t in range(CT)]
            gnw = [prm[:, t, 3:4] for t in range(CT)]
            gnb = [prm[:, t, 4:5] for t in range(CT)]

            # persistent activations (single-pass fp16)
            x16a = persist.tile([P, CT, N], F16, name="x16a")
            k16 = [persist.tile([P, N], F16, name=f"k16_{t}") for t in range(CT)]
            q16 = [persist.tile([P, NQ], F16, name=f"q16_{t}") for t in range(CT)]
            vTa = persist.tile([P, N // P, C], F16, name="vTa")
            vT = [vTa[:, m, :] for m in range(N // P)]
            out_ca = persist.tile([P, CT, NQ], F16, name="out_ca")
            out_c = [out_ca[:, t, :] for t in range(CT)]

            # ---------------- Phase 1: GroupNorm statistics + x16 ----------------
            with (
                tc.tile_pool(name="stat_sb", bufs=1) as stat_sb,
                tc.tile_pool(name="stat_ps", bufs=2, space="PSUM") as stat_ps,
            ):
                stats6 = [stat_sb.tile([P, NCH, 6], F32, name=f"st6_{t}") for t in range(CT)]
                mvs = stat_sb.tile([P, CT, 2], F32, name="mvs")
                sums_acc = stat_sb.tile([P, 2, 2], F32, name="sums_acc")
                for t in range(CT):
                    for hf in range(2):
                        sl = slice(hf * (N // 2), (hf + 1) * (N // 2))
                        nc.sync.dma_start(out=x16a[:, t, sl], in_=x_d[t][:, sl])
                        for c2 in range(NCH // 2):
                            ch = hf * (NCH // 2) + c2
                            nc.vector.bn_stats(
                                out=stats6[t][:, ch, :],
                                in_=x16a[:, t, hf * (N // 2) + c2 * 512:
                                         hf * (N // 2) + (c2 + 1) * 512])
                    nc.vector.bn_aggr(out=mvs[:, t, :], in_=stats6[t])
                # preload the Sqrt table set while ACT is idle (a lazy load
                # would land on the stats critical path)
                preld = stat_sb.tile([P, 1], F32, name="preld")
                nc.scalar.activation(out=preld, in_=eps_t, func=ACTF.Sqrt,
                                     bias=eps_t, scale=1.0)
                nc.gpsimd.dma_start(out=wvt_all, in_=wvt_d[:, :, :])
                nc.gpsimd.dma_start(out=wpt_all, in_=wpt_d[:, :, :])
                # stats2 cols: [mean_t0..3 | ex2_t0..3]
                stats2 = stat_sb.tile([P, 8], F32, name="stats2")
                means = mvs[:, :, 0]
                vars_ = mvs[:, :, 1]
                nc.vector.tensor_copy(stats2[:, 0:4], means)
                nc.vector.tensor_tensor(out=stats2[:, 4:8], in0=means, in1=means, op=ALU.mult)
                nc.vector.tensor_tensor(out=stats2[:, 4:8], in0=stats2[:, 4:8], in1=vars_, op=ALU.add)
                # column sums / 256 -> [1, 8] on partition 0
                ps8 = stat_ps.tile([1, 8], F32, name="ps8")
                nc.tensor.matmul(ps8, inv256, stats2, start=True, stop=True)
                s8 = stat_sb.tile([1, 8], F32, name="s8")
                nc.vector.tensor_copy(s8, ps8)
                # per-group mean and E[x^2]: adjacent-pair sums
                gme = stat_sb.tile([1, 4], F32, name="gme")  # [mu_g0, mu_g1, e_g0, e_g1]
                s8v = s8.rearrange("p (f g two) -> p f g two", f=2, two=2)
                gmev = gme.rearrange("p (f g) -> p f g", f=2)
                nc.vector.tensor_tensor(
                    out=gmev[:, :, :], in0=s8v[:, :, :, 0], in1=s8v[:, :, :, 1], op=ALU.add)
                # broadcast to 128 partitions: [128, 4]
                psb = stat_ps.tile([P, 4], F32, name="psb")
                nc.tensor.matmul(psb, ones_f32[0:1, :], gme, start=True, stop=True)
                mu_e = stat_sb.tile([P, 4], F32, name="mu_e")
                nc.vector.tensor_copy(mu_e, psb)
                mu_bc = mu_e[:, 0:2]
                e_bc = mu_e[:, 2:4]
                var_bc = stat_sb.tile([P, 2], F32, name="var_bc")
                nc.vector.tensor_tensor(out=var_bc, in0=mu_bc, in1=mu_bc, op=ALU.mult)
                nc.vector.tensor_tensor(out=var_bc, in0=e_bc, in1=var_bc, op=ALU.subtract)
                sd = stat_sb.tile([P, 2], F32, name="sd")
                for g in range(G):
                    nc.scalar.activation(out=sd[:, g:g + 1], in_=var_bc[:, g:g + 1],
                                         func=ACTF.Sqrt, bias=eps_t, scale=1.0)
                nc.scalar.activation(out=preld, in_=eps_t, func=ACTF.Exp)
                rstd = stat_sb.tile([P, 2], F32, name="rstd")
                nc.vector.reciprocal(out=rstd, in_=sd)
                # per-channel-tile affine: h = a*x + b (vectorized over tiles:
                # gnw/gnb live strided in prm as [P, CT])
                ab = persist.tile([P, 2, CT], F32, name="ab")
                a4 = ab[:, 0, :]
                b4 = ab[:, 1, :]
                gnw4 = prm[:, :, 3]
                gnb4 = prm[:, :, 4]
                for g in range(G):
                    gs = slice(2 * g, 2 * g + 2)
                    nc.vector.tensor_scalar(
                        out=a4[:, gs], in0=gnw4[:, gs], scalar1=rstd[:, g:g + 1],
                        scalar2=None, op0=ALU.mult)
                    nc.vector.tensor_scalar(
                        out=b4[:, gs], in0=a4[:, gs], scalar1=mu_bc[:, g:g + 1],
                        scalar2=None, op0=ALU.mult)
                nc.vector.tensor_tensor(out=b4, in0=gnb4, in1=b4, op=ALU.subtract)
                a_t = [a4[:, t:t + 1] for t in range(CT)]
                b_t = [b4[:, t:t + 1] for t in range(CT)]

            # ---------------- Phase 2: h + q/k/v convs (from SBUF x16) ----------------
            with (
                tc.tile_pool(name="cq_ps", bufs=2, space="PSUM") as cq_ps,
                tc.tile_pool(name="ck_ps", bufs=2, space="PSUM") as ck_ps,
                tc.tile_pool(name="cv_ps", bufs=2, space="PSUM") as cv_ps,
            ):
                for ch in range(NCH):
                    sl = slice(ch * 512, (ch + 1) * 512)
                    h16 = []
                    for t in range(CT):
                        h16t = h16_pool.tile([P, 512], F16, name="h16", tag="h16")
                        nc.vector.tensor_scalar(
                            out=h16t, in0=x16a[:, t, sl], scalar1=a_t[t], scalar2=b_t[t],
                            op0=ALU.mult, op1=ALU.add)
                        h16.append(h16t)
                    # k conv (and q for first half): single-pass fp16
                    for o in range(CT):
                        osl = slice(o * P, (o + 1) * P)
                        kp = ck_ps.tile([P, 512], F32, name="kp", tag="kp")
                        for t in range(CT):
                            nc.tensor.matmul(
                                kp, wkt[t][:, osl], h16[t],
                                start=(t == 0), stop=(t == CT - 1))
                        nc.scalar.activation(
                            out=k16[o][:, sl], in_=kp, func=ACTF.Identity,
                            bias=bk[o], scale=1.0)
                        if ch < NCH // 2:
                            qp = cq_ps.tile([P, 512], F32, name="qp", tag="qp")
                            for t in range(CT):
                                nc.tensor.matmul(
                                    qp, wqt[t][:, osl], h16[t],
                                    start=(t == 0), stop=(t == CT - 1))
                            nc.scalar.activation(
                                out=q16[o][:, sl], in_=qp, func=ACTF.Identity,
                                bias=bq[o], scale=1.0)
                    # v conv, transposed output (ch 6,7 deferred into phase 3
                    # as PE cover for block 0's softmax chains)
                    if ch < 6:
                        for mb in range(4):
                            m = ch * 4 + mb
                            vp = cv_ps.tile([P, C], F32, name="vp", tag="vp")
                            for t in range(CT):
                                nc.tensor.matmul(
                                    vp, h16[t][:, mb * P:(mb + 1) * P], wvt[t],
                                    start=(t == 0), stop=(t == CT - 1))
                            nc.vector.tensor_copy(vT[m], vp)

            # ---------------- Phase 3: attention ----------------
            with (
                tc.tile_pool(name="att_sb", bufs=1) as att_sb,
                tc.tile_pool(name="p_pool", bufs=2) as p_pool,
                tc.tile_pool(name="pt_pool", bufs=2) as pt_pool,
                tc.tile_pool(name="ot_pool", bufs=2) as ot_pool,
                tc.tile_pool(name="sc_ps", bufs=5, space="PSUM") as sc_ps,
                tc.tile_pool(name="tp_ps", bufs=1, space="PSUM") as tp_ps,
                tc.tile_pool(name="o_ps", bufs=1, space="PSUM") as o_ps,
                tc.tile_pool(name="pp_ps", bufs=1, space="PSUM") as pp_ps,
                tc.tile_pool(name="fin_pool", bufs=4) as fin_pool,
            ):
                def emit_proj_part(g2b, o, pool=None):
                    sl = slice(g2b * 256, (g2b + 1) * 256)
                    pp = (pool.tile([P, 256], F32, name="pp", tag="sp") if pool
                          else pp_ps.tile([P, 256], F32, name="pp", tag="pp"))
                    for t in range(CT):
                        nc.tensor.matmul(
                            pp, wpt[t][:, o * P:(o + 1) * P], out_c[t][:, sl],
                            start=(t == 0), stop=(t == CT - 1))
                    fin = fin_pool.tile([P, 256], F32, name="fin", tag="fin")
                    nc.scalar.activation(
                        out=fin, in_=pp, func=ACTF.Identity, bias=bp[o], scale=1.0)
                    nc.vector.tensor_tensor(
                        out=fin, in0=fin, in1=x16a[:, o, sl], op=ALU.add)
                    nc.gpsimd.dma_start(out=out_d[o][:, sl], in_=fin)

                def emit_scores_half(nb, half, st=None):
                    """One key half (4 chunks of 512), chunk-contiguous matmuls
                    with each chunk's max emitted right after its matmuls."""
                    if half == 0:
                        pt_b = p_pool.tile([P, N], F16, name="pexp", tag="pexp")
                        sums = att_sb.tile([P, 8], F32, name="sums", tag="sums", bufs=2)
                        mx = att_sb.tile([P, 8], F32, name="mx", tag="mx", bufs=2)
                        small = att_sb.tile([P, 4], F32, name="small", tag="small", bufs=2)
                    else:
                        pt_b, sums, mx, small = st
                    negm1, negm, alpha, s_tot = (small[:, i:i + 1] for i in range(4))
                    nsl = slice(nb * P, (nb + 1) * P)
                    sps = []
                    for j in range(4):
                        mch = 4 * half + j
                        msl = slice(mch * 512, (mch + 1) * 512)
                        sp = sc_ps.tile([P, 512], F32, name="sp", tag="sp")
                        for t in range(CT):
                            nc.tensor.matmul(
                                sp, q16[t][:, nsl], k16[t][:, msl],
                                start=(t == 0), stop=(t == CT - 1))
                        nc.vector.reduce_max(out=mx[:, mch:mch + 1], in_=sp, axis=AX)
                        sps.append(sp)
                    if half == 0:
                        nc.vector.reduce_max(out=negm1, in_=mx[:, 0:4], axis=AX, negate=True)
                        for j in range(4):
                            nc.scalar.activation(
                                out=pt_b[:, j * 512:(j + 1) * 512], in_=sps[j],
                                func=ACTF.Exp, bias=negm1, scale=1.0,
                                accum_out=sums[:, j:j + 1])
                        return (pt_b, sums, mx, small)
                    else:
                        nc.vector.reduce_max(out=negm, in_=mx[:, 4:8], axis=AX, negate=True)
                        nc.vector.tensor_tensor(out=negm, in0=negm, in1=negm1, op=ALU.min)
                        nc.vector.tensor_tensor(out=alpha, in0=negm, in1=negm1, op=ALU.subtract)
                        nc.scalar.activation(out=alpha, in_=alpha, func=ACTF.Exp)
                        for j in range(4):
                            nc.scalar.activation(
                                out=pt_b[:, (4 + j) * 512:(5 + j) * 512], in_=sps[j],
                                func=ACTF.Exp, bias=negm, scale=1.0,
                                accum_out=sums[:, 4 + j:5 + j])
                        return (pt_b, sums, mx, small)

                def emit_tail(st):
                    """Rescale A-half by alpha; total sum and reciprocal."""
                    pt_b, sums, mx, small = st
                    negm1, negm, alpha, s_tot = (small[:, i:i + 1] for i in range(4))
                    nc.vector.tensor_scalar_mul(
                        out=pt_b[:, 0:NQ], in0=pt_b[:, 0:NQ], scalar1=alpha)
                    nc.vector.tensor_scalar_mul(
                        out=sums[:, 0:4], in0=sums[:, 0:4], scalar1=alpha)
                    recip = att_sb.tile([P, 1], F32, name="recip", tag="recip", bufs=2)
                    nc.vector.reduce_sum(out=s_tot, in_=sums, axis=AX)
                    nc.vector.reciprocal(out=recip, in_=s_tot)
                    return recip

                def emit_v_deferred(ch):
                    """v-conv for one deferred chunk (h16 recomputed on DVE);
                    vp double-buffered through the po/pp banks."""
                    sl = slice(ch * 512, (ch + 1) * 512)
                    h16 = []
                    for t in range(CT):
                        h16t = h16_pool.tile([P, 512], F16, name="h16", tag="h16")
                        nc.vector.tensor_scalar(
                            out=h16t, in0=x16a[:, t, sl], scalar1=a_t[t], scalar2=b_t[t],
                            op0=ALU.mult, op1=ALU.add)
                        h16.append(h16t)
                    for mb in range(4):
                        m = ch * 4 + mb
                        pool, tg = (o_ps, "po") if mb % 2 else (pp_ps, "pp")
                        vp = pool.tile([P, C], F32, name="vpd", tag=tg)
                        for t in range(CT):
                            nc.tensor.matmul(
                                vp, h16[t][:, mb * P:(mb + 1) * P], wvt[t],
                                start=(t == 0), stop=(t == CT - 1))
                        nc.vector.tensor_copy(vT[m], vp)

                def emit_apply_half(nb, st, po, g2s, tpool=None):
                    """Transpose + attnV for two groups of 8 key tiles,
                    T/copy/V interleaved. Groups 2,3 (B-half of pt_b, which
                    needs no alpha rescale) run first so apply1 does not wait
                    on the previous iteration's rescale."""
                    pt_b = st[0]
                    for g2 in g2s:
                        tp = (tpool.tile([P, 1024], F16, name="tp", tag="sp")
                              if tpool else
                              tp_ps.tile([P, 1024], F16, name="tp", tag="tp"))
                        for j in range(8):
                            mt = 8 * g2 + j
                            nc.tensor.transpose(
                                tp[:, j * P:(j + 1) * P], pt_b[:, mt * P:(mt + 1) * P], ident)
                        ptg = pt_pool.tile([P, 1024], F16, name="ptg", tag="ptg")
                        nc.vector.tensor_copy(ptg, tp)
                        for j in range(8):
                            mt = 8 * g2 + j
                            nc.tensor.matmul(
                                po, ptg[:, j * P:(j + 1) * P], vT[mt],
                                start=(mt == 16), stop=(mt == 15))

                def emit_out(nb, po, recip):
                    """Normalize + transpose out_T back to [c, n]."""
                    nsl = slice(nb * P, (nb + 1) * P)
                    oT = ot_pool.tile([P, C], F16, name="oT", tag="oT")
                    nc.vector.tensor_scalar_mul(out=oT, in0=po, scalar1=recip)
                    tp2 = tp_ps.tile([P, 512], F16, name="tp2", tag="tp")
                    for t in range(CT):
                        nc.tensor.transpose(
                            tp2[:, t * P:(t + 1) * P], oT[:, t * P:(t + 1) * P], ident)
                    tp2v = tp2.rearrange("p (t n) -> p t n", t=CT)
                    nc.vector.tensor_copy(out_ca[:, :, nsl], tp2v)

                # software pipeline across iterations:
                #   iter nb: scoresA(nb) | apply1(nb-1) | scoresB(nb) | apply2(nb-1)
                #   proj for 4-block group g spread across iter 4g+5.
                prev = None         # (st, po, recip) of block nb-1
                for nb in range(NQB + 2):
                    pj = nb - 2     # proj pair-group source block
                    do_proj = pj >= 1 and (pj % 2) == 1
                    gp = (pj // 2) if do_proj else None
                    # the last proj group runs after scores are done: pipeline
                    # its psum through the freed score banks instead of pp_ps
                    pjpool = sc_ps if (do_proj and nb >= NQB + 1) else None
                    if do_proj:
                        emit_proj_part(gp, 0, pjpool)
                    stA = emit_scores_half(nb, 0) if nb < NQB else None
                    if nb == 0:
                        emit_v_deferred(6)
                    # drain phase: scores are done, pipeline the last apply's
                    # transposes through the freed score banks
                    tpool = sc_ps if nb - 1 >= NQB - 1 else None
                    if prev is not None:
                        st_p, recip_p = prev
                        po = o_ps.tile([P, C], F32, name="po", tag="po")
                        emit_apply_half(nb - 1, st_p, po, (2, 3), tpool)
                    if do_proj:
                        emit_proj_part(gp, 1, pjpool)
                    if nb < NQB:
                        stB = emit_scores_half(nb, 1, stA)
                    if nb == 0:
                        emit_v_deferred(7)
                    if do_proj:
                        emit_proj_part(gp, 2, pjpool)
                    if prev is not None:
                        emit_apply_half(nb - 1, st_p, po, (0, 1), tpool)
                        emit_out(nb - 1, po, recip_p)
                    if nb < NQB:
                        recip = emit_tail(stB)
                        prev = (stB, recip)
                    else:
                        prev = None
                    if do_proj:
                        emit_proj_part(gp, 3, pjpool)

    nc.compile()
    return nc


def _prep_shared(gn_w, gn_b, wq, bq, wk, bk, wv, bv, wp, bp):
    f32 = np.float32
    s = f32(math.sqrt(512.0))
    def pack(wT):  # [C, C] -> [P, CT, C] partition-major
        return np.ascontiguousarray(wT.reshape(CT, P, C).transpose(1, 0, 2))

    prm = np.zeros((P, CT, 6), dtype=f32)
    prm[:, :, 0] = (bq.astype(f32) * s).reshape(CT, P).T
    prm[:, :, 1] = bk.astype(f32).reshape(CT, P).T
    # v-bias folded into the proj bias: attn rows sum to 1, so
    # proj(attn@v + bv) = proj(attn@v0) + (wp@bv + bp)
    bp2 = bp.astype(f32) + wp.astype(f32) @ bv.astype(f32)
    prm[:, :, 2] = bp2.reshape(CT, P).T
    prm[:, :, 3] = gn_w.astype(f32).reshape(CT, P).T
    prm[:, :, 4] = gn_b.astype(f32).reshape(CT, P).T
    shared = {
        "wqt": pack((wq.T * s).astype(f32)).astype(np.float16),
        "wkt": pack(wk.T.astype(f32)).astype(np.float16),
        "wvt": pack(wv.T.astype(f32)).astype(np.float16),
        "wpt": pack(wp.T.astype(f32)).astype(np.float16),
        "prm": prm,
    }
    return shared


def _make_in_maps(inputs):
    x = np.asarray(inputs["x"], dtype=np.float32)
    args = [np.asarray(inputs[k], dtype=np.float32) for k in
            ("gn_w", "gn_b", "wq", "bq", "wk", "bk", "wv", "bv", "wp", "bp")]
    shared = _prep_shared(*args)
    in_maps = []
    for core in range(8):
        b, half = core // 2, core % 2
        xb = x[b].reshape(C, N)
        if half:
            xb = np.concatenate([xb[:, NQ:], xb[:, :NQ]], axis=1)
        m = dict(shared)
        m["x"] = np.ascontiguousarray(xb.reshape(CT, P, N)).astype(np.float16)
        in_maps.append(m)
    return in_maps


def kernel(x, gn_w, gn_b, wq, bq, wk, bk, wv, bv, wp, bp):
    global _CACHED_NC
    if _CACHED_NC is None:
        _CACHED_NC = build_nc()
    nc = _CACHED_NC

    in_maps = _make_in_maps(dict(x=x, gn_w=gn_w, gn_b=gn_b, wq=wq, bq=bq, wk=wk,
                                 bk=bk, wv=wv, bv=bv, wp=wp, bp=bp))
    res = run_bass_kernel_spmd(nc, in_maps, core_ids=list(range(8)))

    y = np.empty((B, C, N), dtype=np.float32)
    for core in range(8):
        b, half = core // 2, core % 2
        y[b][:, half * NQ:(half + 1) * NQ] = res.results[core]["out"].reshape(C, NQ)
    return y.reshape(B, C, H, W)


# revision 10
# speedup vs baseline: 1.1284x; 1.1284x over previous
"""AttnBlock kernel for 8 Trainium2 NeuronCores — single-pass fp16, v3.

Sharding: 8 cores = 4 examples x 2 query-halves (columns rotated per core so
its half comes first; softmax over keys is permutation invariant). No
cross-core communication.

v3 structure: x is DMA'd once (phase 1), stats via bn_stats, and an fp16 copy
of x is kept in SBUF for the convs and the residual (saves 16 MB of HBM
re-reads). Convs and scores are single-pass fp16 (validated rel_err ~1e-2 on
the fixed inputs). Attention uses a 2-half online softmax with chunk-contiguous
score matmuls (per-chunk maxes overlap the score stream), 5 rotating score
PSUM banks so pass B starts before all of pass A is consumed, interleaved
transpose/attnV groups, and the proj of each 4-block group spread across the
following iteration.
"""

import math

import numpy as np

import concourse.bacc as bacc
import concourse.mybir as mybir
import concourse.tile as tile
from concourse.bass_utils import run_bass_kernel_spmd
from concourse.masks import make_identity

F32 = mybir.dt.float32
F16 = mybir.dt.float16

B, C, H, W = 4, 512, 64, 64
N = H * W            # 4096 key positions
NQ = N // 2          # 2048 query positions per core
P = 128              # partitions
CT = C // P          # 4 channel tiles
NCH = N // 512       # 8 key chunks of 512
NQB = NQ // P        # 16 query blocks of 128
G = 2                # groupnorm groups
EPS = 1e-6
AX = mybir.AxisListType.X
ALU = mybir.AluOpType
ACTF = mybir.ActivationFunctionType

_CACHED_NC = None


def build_nc(loop_r: int = 1):
    nc = bacc.Bacc("TRN2", target_bir_lowering=False)

    x_d = nc.dram_tensor("x", [CT, P, N], F32, kind="ExternalInput")
    wqt_d = nc.dram_tensor("wqt", [P, CT, C], F16, kind="ExternalInput")  # [p, t, o], q scaled by sqrt(C)
    wkt_d = nc.dram_tensor("wkt", [P, CT, C], F16, kind="ExternalInput")
    wvt_d = nc.dram_tensor("wvt", [P, CT, C], F16, kind="ExternalInput")
    wpt_d = nc.dram_tensor("wpt", [P, CT, C], F16, kind="ExternalInput")
    # per-channel params packed: [p, t, (bq, bk, bp', gnw, gnb, pad)]
    # bp' = bp + wp @ bv  (v-bias folded into proj bias; attn rows sum to 1)
    prm_d = nc.dram_tensor("prm", [P, CT, 6], F32, kind="ExternalInput")
    out_d = nc.dram_tensor("out", [CT, P, NQ], F32, kind="ExternalOutput")

    import contextlib

    with tile.TileContext(nc) as tc:
        loop_ctx = tc.For_i(0, loop_r, 1) if loop_r > 1 else contextlib.nullcontext()
        with (
            loop_ctx,
            tc.tile_pool(name="singles", bufs=1) as singles,
            tc.tile_pool(name="persist", bufs=1) as persist,
            tc.tile_pool(name="h16_pool", bufs=8) as h16_pool,
        ):
            ident = singles.tile([P, P], F16, name="ident")
            make_identity(nc, ident)
            ones_f32 = singles.tile([P, P], F32, name="ones_f32")
            nc.vector.memset(ones_f32, 1.0)
            inv256 = singles.tile([P, 1], F32, name="inv256")
            nc.vector.memset(inv256, 1.0 / 256.0)
            eps_t = singles.tile([P, 1], F32, name="eps_t")
            nc.vector.memset(eps_t, EPS)
            inv4096 = singles.tile([P, 1], F32, name="inv4096")
            nc.vector.memset(inv4096, 1.0 / N)

            # weights and per-channel params: one packed DMA each
            wqt_all = persist.tile([P, CT, C], F16, name="wqt_all")
            wkt_all = persist.tile([P, CT, C], F16, name="wkt_all")
            wvt_all = persist.tile([P, CT, C], F16, name="wvt_all")
            wpt_all = persist.tile([P, CT, C], F16, name="wpt_all")
            prm = persist.tile([P, CT, 6], F32, name="prm")
            # prm loads early (needed for the stats tail); all weights queue on
            # the sync queue behind the phase-1 x stream so they don't steal
            # HBM bandwidth from it — they arrive just in time for phase 2
            nc.gpsimd.dma_start(out=prm, in_=prm_d[:, :, :])
            nc.gpsimd.dma_start(out=wkt_all, in_=wkt_d[:, :, :])
            nc.gpsimd.dma_start(out=wqt_all, in_=wqt_d[:, :, :])
            wqt = [wqt_all[:, t, :] for t in range(CT)]
            wkt = [wkt_all[:, t, :] for t in range(CT)]
            wvt = [wvt_all[:, t, :] for t in range(CT)]
            wpt = [wpt_all[:, t, :] for t in range(CT)]
            bq = [prm[:, t, 0:1] for t in range(CT)]
            bk = [prm[:, t, 1:2] for t in range(CT)]
            bp = [prm[:, t, 2:3] for t in range(CT)]
            gnw = [prm[:, t, 3:4] for t in range(CT)]
            gnb = [prm[:, t, 4:5] for t in range(CT)]

            # persistent activations (single-pass fp16)
            x16a = persist.tile([P, CT, N], F16, name="x16a")
            k16 = [persist.tile([P, N], F16, name=f"k16_{t}") for t in range(CT)]
            q16 = [persist.tile([P, NQ], F16, name=f"q16_{t}") for t in range(CT)]
            vTa = persist.tile([P, N // P, C], F16, name="vTa")
            vT = [vTa[:, m, :] for m in range(N // P)]
            out_ca = persist.tile([P, CT, NQ], F16, name="out_ca")
            out_c = [out_ca[:, t, :] for t in range(CT)]

            # ---------------- Phase 1: GroupNorm statistics + x16 ----------------
            with (
                tc.tile_pool(name="stat_sb", bufs=1) as stat_sb,
                tc.tile_pool(name="stat_ps", bufs=2, space="PSUM") as stat_ps,
            ):
                stats6 = [stat_sb.tile([P, NCH, 6], F32, name=f"st6_{t}") for t in range(CT)]
                mvs = stat_sb.tile([P, CT, 2], F32, name="mvs")
                sums_acc = stat_sb.tile([P, 2, 2], F32, name="sums_acc")
                for t in range(CT):
                    for hf in range(2):
                        sl = slice(hf * (N // 2), (hf + 1) * (N // 2))
                        xb = stat_sb.tile([P, N // 2], F32, name="xbig", tag="xbig", bufs=3)
                        nc.gpsimd.dma_start(out=xb, in_=x_d[t][:, sl])
                        for c2 in range(NCH // 2):
                            ch = hf * (NCH // 2) + c2
                            nc.vector.bn_stats(
                                out=stats6[t][:, ch, :], in_=xb[:, c2 * 512:(c2 + 1) * 512])
                        nc.scalar.activation(
                            out=x16a[:, t, sl], in_=xb, func=ACTF.Identity, scale=1.0)
                    nc.vector.bn_aggr(out=mvs[:, t, :], in_=stats6[t])
                # preload the Sqrt table set while ACT is idle (a lazy load
                # would land on the stats critical path)
                preld = stat_sb.tile([P, 1], F32, name="preld")
                nc.scalar.activation(out=preld, in_=eps_t, func=ACTF.Sqrt,
                                     bias=eps_t, scale=1.0)
                nc.gpsimd.dma_start(out=wvt_all, in_=wvt_d[:, :, :])
                nc.gpsimd.dma_start(out=wpt_all, in_=wpt_d[:, :, :])
                # stats2 cols: [mean_t0..3 | ex2_t0..3]
                stats2 = stat_sb.tile([P, 8], F32, name="stats2")
                means = mvs[:, :, 0]
                vars_ = mvs[:, :, 1]
                nc.vector.tensor_copy(stats2[:, 0:4], means)
                nc.vector.tensor_tensor(out=stats2[:, 4:8], in0=means, in1=means, op=ALU.mult)
                nc.vector.tensor_tensor(out=stats2[:, 4:8], in0=stats2[:, 4:8], in1=vars_, op=ALU.add)
                # column sums / 256 -> [1, 8] on partition 0
                ps8 = stat_ps.tile([1, 8], F32, name="ps8")
                nc.tensor.matmul(ps8, inv256, stats2, start=True, stop=True)
                s8 = stat_sb.tile([1, 8], F32, name="s8")
                nc.vector.tensor_copy(s8, ps8)
                # per-group mean and E[x^2]: adjacent-pair sums
                gme = stat_sb.tile([1, 4], F32, name="gme")  # [mu_g0, mu_g1, e_g0, e_g1]
                s8v = s8.rearrange("p (f g two) -> p f g two", f=2, two=2)
                gmev = gme.rearrange("p (f g) -> p f g", f=2)
                nc.vector.tensor_tensor(
                    out=gmev[:, :, :], in0=s8v[:, :, :, 0], in1=s8v[:, :, :, 1], op=ALU.add)
                # broadcast to 128 partitions: [128, 4]
                psb = stat_ps.tile([P, 4], F32, name="psb")
                nc.tensor.matmul(psb, ones_f32[0:1, :], gme, start=True, stop=True)
                mu_e = stat_sb.tile([P, 4], F32, name="mu_e")
                nc.vector.tensor_copy(mu_e, psb)
                mu_bc = mu_e[:, 0:2]
                e_bc = mu_e[:, 2:4]
                var_bc = stat_sb.tile([P, 2], F32, name="var_bc")
                nc.vector.tensor_tensor(out=var_bc, in0=mu_bc, in1=mu_bc, op=ALU.mult)
                nc.vector.tensor_tensor(out=var_bc, in0=e_bc, in1=var_bc, op=ALU.subtract)
                sd = stat_sb.tile([P, 2], F32, name="sd")
                for g in range(G):
                    nc.scalar.activation(out=sd[:, g:g + 1], in_=var_bc[:, g:g + 1],
                                         func=ACTF.Sqrt, bias=eps_t, scale=1.0)
                nc.scalar.activation(out=preld, in_=eps_t, func=ACTF.Exp)
                rstd = stat_sb.tile([P, 2], F32, name="rstd")
                nc.vector.reciprocal(out=rstd, in_=sd)
                # per-channel-tile affine: h = a*x + b (vectorized over tiles:
                # gnw/gnb live strided in prm as [P, CT])
                ab = persist.tile([P, 2, CT], F32, name="ab")
                a4 = ab[:, 0, :]
                b4 = ab[:, 1, :]
                gnw4 = prm[:, :, 3]
                gnb4 = prm[:, :, 4]
                for g in range(G):
                    gs = slice(2 * g, 2 * g + 2)
                    nc.vector.tensor_scalar(
                        out=a4[:, gs], in0=gnw4[:, gs], scalar1=rstd[:, g:g + 1],
                        scalar2=None, op0=ALU.mult)
                    nc.vector.tensor_scalar(
                        out=b4[:, gs], in0=a4[:, gs], scalar1=mu_bc[:, g:g + 1],
                        scalar2=None, op0=ALU.mult)
                nc.vector.tensor_tensor(out=b4, in0=gnb4, in1=b4, op=ALU.subtract)
                a_t = [a4[:, t:t + 1] for t in range(CT)]
                b_t = [b4[:, t:t + 1] for t in range(CT)]

            # ---------------- Phase 2: h + q/k/v convs (from SBUF x16) ----------------
            with (
                tc.tile_pool(name="cq_ps", bufs=2, space="PSUM") as cq_ps,
                tc.tile_pool(name="ck_ps", bufs=2, space="PSUM") as ck_ps,
                tc.tile_pool(name="cv_ps", bufs=2, space="PSUM") as cv_ps,
            ):
                for ch in range(NCH):
                    sl = slice(ch * 512, (ch + 1) * 512)
                    h16 = []
                    for t in range(CT):
                        h16t = h16_pool.tile([P, 512], F16, name="h16", tag="h16")
                        nc.vector.tensor_scalar(
                            out=h16t, in0=x16a[:, t, sl], scalar1=a_t[t], scalar2=b_t[t],
                            op0=ALU.mult, op1=ALU.add)
                        h16.append(h16t)
                    # k conv (and q for first half): single-pass fp16
                    for o in range(CT):
                        osl = slice(o * P, (o + 1) * P)
                        kp = ck_ps.tile([P, 512], F32, name="kp", tag="kp")
                        for t in range(CT):
                            nc.tensor.matmul(
                                kp, wkt[t][:, osl], h16[t],
                                start=(t == 0), stop=(t == CT - 1))
                        nc.scalar.activation(
                            out=k16[o][:, sl], in_=kp, func=ACTF.Identity,
                            bias=bk[o], scale=1.0)
                        if ch < NCH // 2:
                            qp = cq_ps.tile([P, 512], F32, name="qp", tag="qp")
                            for t in range(CT):
                                nc.tensor.matmul(
                                    qp, wqt[t][:, osl], h16[t],
                                    start=(t == 0), stop=(t == CT - 1))
                            nc.scalar.activation(
                                out=q16[o][:, sl], in_=qp, func=ACTF.Identity,
                                bias=bq[o], scale=1.0)
                    # v conv, transposed output (ch 6,7 deferred into phase 3
                    # as PE cover for block 0's softmax chains)
                    if ch < 6:
                        for mb in range(4):
                            m = ch * 4 + mb
                            vp = cv_ps.tile([P, C], F32, name="vp", tag="vp")
                            for t in range(CT):
                                nc.tensor.matmul(
                                    vp, h16[t][:, mb * P:(mb + 1) * P], wvt[t],
                                    start=(t == 0), stop=(t == CT - 1))
                            nc.vector.tensor_copy(vT[m], vp)

            # ---------------- Phase 3: attention ----------------
            with (
                tc.tile_pool(name="att_sb", bufs=1) as att_sb,
                tc.tile_pool(name="p_pool", bufs=2) as p_pool,
                tc.tile_pool(name="pt_pool", bufs=2) as pt_pool,
                tc.tile_pool(name="ot_pool", bufs=2) as ot_pool,
                tc.tile_pool(name="sc_ps", bufs=5, space="PSUM") as sc_ps,
                tc.tile_pool(name="tp_ps", bufs=1, space="PSUM") as tp_ps,
                tc.tile_pool(name="o_ps", bufs=1, space="PSUM") as o_ps,
                tc.tile_pool(name="pp_ps", bufs=1, space="PSUM") as pp_ps,
                tc.tile_pool(name="fin_pool", bufs=4) as fin_pool,
            ):
                def emit_proj_part(g2b, o, pool=None):
                    sl = slice(g2b * 256, (g2b + 1) * 256)
                    pp = (pool.tile([P, 256], F32, name="pp", tag="sp") if pool
                          else pp_ps.tile([P, 256], F32, name="pp", tag="pp"))
                    for t in range(CT):
                        nc.tensor.matmul(
                            pp, wpt[t][:, o * P:(o + 1) * P], out_c[t][:, sl],
                            start=(t == 0), stop=(t == CT - 1))
                    fin = fin_pool.tile([P, 256], F32, name="fin", tag="fin")
                    nc.scalar.activation(
                        out=fin, in_=pp, func=ACTF.Identity, bias=bp[o], scale=1.0)
                    nc.vector.tensor_tensor(
                        out=fin, in0=fin, in1=x16a[:, o, sl], op=ALU.add)
                    nc.gpsimd.dma_start(out=out_d[o][:, sl], in_=fin)

                def emit_scores_half(nb, half, st=None):
                    """One key half (4 chunks of 512), chunk-contiguous matmuls
                    with each chunk's max emitted right after its matmuls."""
                    if half == 0:
                        pt_b = p_pool.tile([P, N], F16, name="pexp", tag="pexp")
                        sums = att_sb.tile([P, 8], F32, name="sums", tag="sums", bufs=2)
                        mx = att_sb.tile([P, 8], F32, name="mx", tag="mx", bufs=2)
                        small = att_sb.tile([P, 4], F32, name="small", tag="small", bufs=2)
                    else:
                        pt_b, sums, mx, small = st
                    negm1, negm, alpha, s_tot = (small[:, i:i + 1] for i in range(4))
                    nsl = slice(nb * P, (nb + 1) * P)
                    sps = []
                    for j in range(4):
                        mch = 4 * half + j
                        msl = slice(mch * 512, (mch + 1) * 512)
                        sp = sc_ps.tile([P, 512], F32, name="sp", tag="sp")
                        for t in range(CT):
                            nc.tensor.matmul(
                                sp, q16[t][:, nsl], k16[t][:, msl],
                                start=(t == 0), stop=(t == CT - 1))
                        nc.vector.reduce_max(out=mx[:, mch:mch + 1], in_=sp, axis=AX)
                        sps.append(sp)
                    if half == 0:
                        nc.vector.reduce_max(out=negm1, in_=mx[:, 0:4], axis=AX, negate=True)
                        for j in range(4):
                            nc.scalar.activation(
                                out=pt_b[:, j * 512:(j + 1) * 512], in_=sps[j],
                                func=ACTF.Exp, bias=negm1, scale=1.0,
                                accum_out=sums[:, j:j + 1])
                        return (pt_b, sums, mx, small)
                    else:
                        nc.vector.reduce_max(out=negm, in_=mx[:, 4:8], axis=AX, negate=True)
                        nc.vector.tensor_tensor(out=negm, in0=negm, in1=negm1, op=ALU.min)
                        nc.vector.tensor_tensor(out=alpha, in0=negm, in1=negm1, op=ALU.subtract)
                        nc.scalar.activation(out=alpha, in_=alpha, func=ACTF.Exp)
                        for j in range(4):
                            nc.scalar.activation(
                                out=pt_b[:, (4 + j) * 512:(5 + j) * 512], in_=sps[j],
                                func=ACTF.Exp, bias=negm, scale=1.0,
                                accum_out=sums[:, 4 + j:5 + j])
                        return (pt_b, sums, mx, small)

                def emit_tail(st):
                    """Rescale A-half by alpha; total sum and reciprocal."""
                    pt_b, sums, mx, small = st
                    negm1, negm, alpha, s_tot = (small[:, i:i + 1] for i in range(4))
                    nc.vector.tensor_scalar_mul(
                        out=pt_b[:, 0:NQ], in0=pt_b[:, 0:NQ], scalar1=alpha)
                    nc.vector.tensor_scalar_mul(
                        out=sums[:, 0:4], in0=sums[:, 0:4], scalar1=alpha)
                    recip = att_sb.tile([P, 1], F32, name="recip", tag="recip", bufs=2)
                    nc.vector.reduce_sum(out=s_tot, in_=sums, axis=AX)
                    nc.vector.reciprocal(out=recip, in_=s_tot)
                    return recip

                def emit_v_deferred(ch):
                    """v-conv for one deferred chunk (h16 recomputed on DVE);
                    vp double-buffered through the po/pp banks."""
                    sl = slice(ch * 512, (ch + 1) * 512)
                    h16 = []
                    for t in range(CT):
                        h16t = h16_pool.tile([P, 512], F16, name="h16", tag="h16")
                        nc.vector.tensor_scalar(
                            out=h16t, in0=x16a[:, t, sl], scalar1=a_t[t], scalar2=b_t[t],
                            op0=ALU.mult, op1=ALU.add)
                        h16.append(h16t)
                    for mb in range(4):
                        m = ch * 4 + mb
                        pool, tg = (o_ps, "po") if mb % 2 else (pp_ps, "pp")
                        vp = pool.tile([P, C], F32, name="vpd", tag=tg)
                        for t in range(CT):
                            nc.tensor.matmul(
                                vp, h16[t][:, mb * P:(mb + 1) * P], wvt[t],
                                start=(t == 0), stop=(t == CT - 1))
                        nc.vector.tensor_copy(vT[m], vp)

                def emit_apply_half(nb, st, po, g2s, tpool=None):
                    """Transpose + attnV for two groups of 8 key tiles,
                    T/copy/V interleaved. Groups 2,3 (B-half of pt_b, which
                    needs no alpha rescale) run first so apply1 does not wait
                    on the previous iteration's rescale."""
                    pt_b = st[0]
                    for g2 in g2s:
                        tp = (tpool.tile([P, 1024], F16, name="tp", tag="sp")
                              if tpool else
                              tp_ps.tile([P, 1024], F16, name="tp", tag="tp"))
                        for j in range(8):
                            mt = 8 * g2 + j
                            nc.tensor.transpose(
                                tp[:, j * P:(j + 1) * P], pt_b[:, mt * P:(mt + 1) * P], ident)
                        ptg = pt_pool.tile([P, 1024], F16, name="ptg", tag="ptg")
                        nc.vector.tensor_copy(ptg, tp)
                        for j in range(8):
                            mt = 8 * g2 + j
                            nc.tensor.matmul(
                                po, ptg[:, j * P:(j + 1) * P], vT[mt],
                                start=(mt == 16), stop=(mt == 15))

                def emit_out(nb, po, recip):
                    """Normalize + transpose out_T back to [c, n]."""
                    nsl = slice(nb * P, (nb + 1) * P)
                    oT = ot_pool.tile([P, C], F16, name="oT", tag="oT")
                    nc.vector.tensor_scalar_mul(out=oT, in0=po, scalar1=recip)
                    tp2 = tp_ps.tile([P, 512], F16, name="tp2", tag="tp")
                    for t in range(CT):
                        nc.tensor.transpose(
                            tp2[:, t * P:(t + 1) * P], oT[:, t * P:(t + 1) * P], ident)
                    tp2v = tp2.rearrange("p (t n) -> p t n", t=CT)
                    nc.vector.tensor_copy(out_ca[:, :, nsl], tp2v)

                # software pipeline across iterations:
                #   iter nb: scoresA(nb) | apply1(nb-1) | scoresB(nb) | apply2(nb-1)
                #   proj for 4-block group g spread across iter 4g+5.
                prev = None         # (st, po, recip) of block nb-1
                for nb in range(NQB + 2):
                    pj = nb - 2     # proj pair-group source block
                    do_proj = pj >= 1 and (pj % 2) == 1
                    gp = (pj // 2) if do_proj else None
                    # the last proj group runs after scores are done: pipeline
                    # its psum through the freed score banks instead of pp_ps
                    pjpool = sc_ps if (do_proj and nb >= NQB + 1) else None
                    if do_proj:
                        emit_proj_part(gp, 0, pjpool)
                    stA = emit_scores_half(nb, 0) if nb < NQB else None
                    if nb == 0:
                        emit_v_deferred(6)
                    # drain phase: scores are done, pipeline the last apply's
                    # transposes through the freed score banks
                    tpool = sc_ps if nb - 1 >= NQB - 1 else None
                    if prev is not None:
                        st_p, recip_p = prev
                        po = o_ps.tile([P, C], F32, name="po", tag="po")
                        emit_apply_half(nb - 1, st_p, po, (2, 3), tpool)
                    if do_proj:
                        emit_proj_part(gp, 1, pjpool)
                    if nb < NQB:
                        stB = emit_scores_half(nb, 1, stA)
                    if nb == 0:
                        emit_v_deferred(7)
                    if do_proj:
                        emit_proj_part(gp, 2, pjpool)
                    if prev is not None:
                        emit_apply_half(nb - 1, st_p, po, (0, 1), tpool)
                        emit_out(nb - 1, po, recip_p)
                    if nb < NQB:
                        recip = emit_tail(stB)
                        prev = (stB, recip)
                    else:
                        prev = None
                    if do_proj:
                        emit_proj_part(gp, 3, pjpool)

    nc.compile()
    return nc


def _prep_shared(gn_w, gn_b, wq, bq, wk, bk, wv, bv, wp, bp):
    f32 = np.float32
    s = f32(math.sqrt(512.0))
    def pack(wT):  # [C, C] -> [P, CT, C] partition-major
        return np.ascontiguousarray(wT.reshape(CT, P, C).transpose(1, 0, 2))

    prm = np.zeros((P, CT, 6), dtype=f32)
    prm[:, :, 0] = (bq.astype(f32) * s).reshape(CT, P).T
    prm[:, :, 1] = bk.astype(f32).reshape(CT, P).T
    # v-bias folded into the proj bias: attn rows sum to 1, so
    # proj(attn@v + bv) = proj(attn@v0) + (wp@bv + bp)
    bp2 = bp.astype(f32) + wp.astype(f32) @ bv.astype(f32)
    prm[:, :, 2] = bp2.reshape(CT, P).T
    prm[:, :, 3] = gn_w.astype(f32).reshape(CT, P).T
    prm[:, :, 4] = gn_b.astype(f32).reshape(CT, P).T
    shared = {
        "wqt": pack((wq.T * s).astype(f32)).astype(np.float16),
        "wkt": pack(wk.T.astype(f32)).astype(np.float16),
        "wvt": pack(wv.T.astype(f32)).astype(np.float16),
        "wpt": pack(wp.T.astype(f32)).astype(np.float16),
        "prm": prm,
    }
    return shared


def _make_in_maps(inputs):
    x = np.asarray(inputs["x"], dtype=np.float32)
    args = [np.asarray(inputs[k], dtype=np.float32) for k in
            ("gn_w", "gn_b", "wq", "bq", "wk", "bk", "wv", "bv", "wp", "bp")]
    shared = _prep_shared(*args)
    in_maps = []
    for core in range(8):
        b, half = core // 2, core % 2
        xb = x[b].reshape(C, N)
        if half:
            xb = np.concatenate([xb[:, NQ:], xb[:, :NQ]], axis=1)
        m = dict(shared)
        m["x"] = np.ascontiguousarray(xb.reshape(CT, P, N))
        in_maps.append(m)
    return in_maps


def kernel(x, gn_w, gn_b, wq, bq, wk, bk, wv, bv, wp, bp):
    global _CACHED_NC
    if _CACHED_NC is None:
        _CACHED_NC = build_nc()
    nc = _CACHED_NC

    in_maps = _make_in_maps(dict(x=x, gn_w=gn_w, gn_b=gn_b, wq=wq, bq=bq, wk=wk,
                                 bk=bk, wv=wv, bv=bv, wp=wp, bp=bp))
    res = run_bass_kernel_spmd(nc, in_maps, core_ids=list(range(8)))

    y = np.empty((B, C, N), dtype=np.float32)
    for core in range(8):
        b, half = core // 2, core % 2
        y[b][:, half * NQ:(half + 1) * NQ] = res.results[core]["out"].reshape(C, NQ)
    return y.reshape(B, C, H, W)


# revision 11
# speedup vs baseline: 1.1807x; 1.0463x over previous
"""AttnBlock kernel for 8 Trainium2 NeuronCores — single-pass fp16, v3.

Sharding: 8 cores = 4 examples x 2 query-halves (columns rotated per core so
its half comes first; softmax over keys is permutation invariant). No
cross-core communication.

v3 structure: x is DMA'd once (phase 1), stats via bn_stats, and an fp16 copy
of x is kept in SBUF for the convs and the residual (saves 16 MB of HBM
re-reads). Convs and scores are single-pass fp16 (validated rel_err ~1e-2 on
the fixed inputs). Attention uses a 2-half online softmax with chunk-contiguous
score matmuls (per-chunk maxes overlap the score stream), 5 rotating score
PSUM banks so pass B starts before all of pass A is consumed, interleaved
transpose/attnV groups, and the proj of each 4-block group spread across the
following iteration.
"""

import math

import numpy as np

import concourse.bacc as bacc
import concourse.mybir as mybir
import concourse.tile as tile
from concourse.bass_utils import run_bass_kernel_spmd
from concourse.masks import make_identity

F32 = mybir.dt.float32
F16 = mybir.dt.float16

B, C, H, W = 4, 512, 64, 64
N = H * W            # 4096 key positions
NQ = N // 2          # 2048 query positions per core
P = 128              # partitions
CT = C // P          # 4 channel tiles
NCH = N // 512       # 8 key chunks of 512
NQB = NQ // P        # 16 query blocks of 128
G = 2                # groupnorm groups
EPS = 1e-6
AX = mybir.AxisListType.X
ALU = mybir.AluOpType
ACTF = mybir.ActivationFunctionType

_CACHED_NC = None


def build_nc(loop_r: int = 1):
    nc = bacc.Bacc("TRN2", target_bir_lowering=False)

    x_d = nc.dram_tensor("x", [CT, P, N], F32, kind="ExternalInput")
    wqt_d = nc.dram_tensor("wqt", [P, CT, C], F16, kind="ExternalInput")  # [p, t, o], q scaled by sqrt(C)
    wkt_d = nc.dram_tensor("wkt", [P, CT, C], F16, kind="ExternalInput")
    wvt_d = nc.dram_tensor("wvt", [P, CT, C], F16, kind="ExternalInput")
    wpt_d = nc.dram_tensor("wpt", [P, CT, C], F16, kind="ExternalInput")
    # per-channel params packed: [p, t, (bq, bk, bp', gnw, gnb, pad)]
    # bp' = bp + wp @ bv  (v-bias folded into proj bias; attn rows sum to 1)
    prm_d = nc.dram_tensor("prm", [P, CT, 6], F32, kind="ExternalInput")
    out_d = nc.dram_tensor("out", [CT, P, NQ], F32, kind="ExternalOutput")

    import contextlib

    with tile.TileContext(nc) as tc:
        loop_ctx = tc.For_i(0, loop_r, 1) if loop_r > 1 else contextlib.nullcontext()
        with (
            loop_ctx,
            tc.tile_pool(name="singles", bufs=1) as singles,
            tc.tile_pool(name="persist", bufs=1) as persist,
            tc.tile_pool(name="h16_pool", bufs=8) as h16_pool,
        ):
            ident = singles.tile([P, P], F16, name="ident")
            make_identity(nc, ident)
            ones_f32 = singles.tile([P, P], F32, name="ones_f32")
            nc.vector.memset(ones_f32, 1.0)
            inv256 = singles.tile([P, 1], F32, name="inv256")
            nc.vector.memset(inv256, 1.0 / 256.0)
            eps_t = singles.tile([P, 1], F32, name="eps_t")
            nc.vector.memset(eps_t, EPS)
            inv4096 = singles.tile([P, 1], F32, name="inv4096")
            nc.vector.memset(inv4096, 1.0 / N)

            # weights and per-channel params: one packed DMA each
            wqt_all = persist.tile([P, CT, C], F16, name="wqt_all")
            wkt_all = persist.tile([P, CT, C], F16, name="wkt_all")
            wvt_all = persist.tile([P, CT, C], F16, name="wvt_all")
            wpt_all = persist.tile([P, CT, C], F16, name="wpt_all")
            prm = persist.tile([P, CT, 6], F32, name="prm")
            # prm loads early (needed for the stats tail); all weights queue on
            # the sync queue behind the phase-1 x stream so they don't steal
            # HBM bandwidth from it — they arrive just in time for phase 2
            nc.gpsimd.dma_start(out=prm, in_=prm_d[:, :, :])
            nc.gpsimd.dma_start(out=wkt_all, in_=wkt_d[:, :, :])
            nc.gpsimd.dma_start(out=wqt_all, in_=wqt_d[:, :, :])
            wqt = [wqt_all[:, t, :] for t in range(CT)]
            wkt = [wkt_all[:, t, :] for t in range(CT)]
            wvt = [wvt_all[:, t, :] for t in range(CT)]
            wpt = [wpt_all[:, t, :] for t in range(CT)]
            bq = [prm[:, t, 0:1] for t in range(CT)]
            bk = [prm[:, t, 1:2] for t in range(CT)]
            bp = [prm[:, t, 2:3] for t in range(CT)]
            gnw = [prm[:, t, 3:4] for t in range(CT)]
            gnb = [prm[:, t, 4:5] for t in range(CT)]

            # persistent activations (single-pass fp16)
            x16a = persist.tile([P, CT, N], F16, name="x16a")
            k16 = [persist.tile([P, N], F16, name=f"k16_{t}") for t in range(CT)]
            q16 = [persist.tile([P, NQ], F16, name=f"q16_{t}") for t in range(CT)]
            vTa = persist.tile([P, N // P, C], F16, name="vTa")
            vT = [vTa[:, m, :] for m in range(N // P)]
            out_ca = persist.tile([P, CT, NQ], F16, name="out_ca")
            out_c = [out_ca[:, t, :] for t in range(CT)]

            # ---------------- Phase 1: GroupNorm statistics + x16 ----------------
            with (
                tc.tile_pool(name="stat_sb", bufs=1) as stat_sb,
                tc.tile_pool(name="stat_ps", bufs=2, space="PSUM") as stat_ps,
            ):
                stats6 = [stat_sb.tile([P, NCH, 6], F32, name=f"st6_{t}") for t in range(CT)]
                mvs = stat_sb.tile([P, CT, 2], F32, name="mvs")
                sums_acc = stat_sb.tile([P, 2, 2], F32, name="sums_acc")
                for t in range(CT):
                    for hf in range(2):
                        sl = slice(hf * (N // 2), (hf + 1) * (N // 2))
                        xb = stat_sb.tile([P, N // 2], F32, name="xbig", tag="xbig", bufs=3)
                        nc.sync.dma_start(out=xb, in_=x_d[t][:, sl])
                        for c2 in range(NCH // 2):
                            ch = hf * (NCH // 2) + c2
                            nc.vector.bn_stats(
                                out=stats6[t][:, ch, :], in_=xb[:, c2 * 512:(c2 + 1) * 512])
                        nc.scalar.activation(
                            out=x16a[:, t, sl], in_=xb, func=ACTF.Identity, scale=1.0)
                    nc.vector.bn_aggr(out=mvs[:, t, :], in_=stats6[t])
                # preload the Sqrt table set while ACT is idle (a lazy load
                # would land on the stats critical path)
                preld = stat_sb.tile([P, 1], F32, name="preld")
                nc.scalar.activation(out=preld, in_=eps_t, func=ACTF.Sqrt,
                                     bias=eps_t, scale=1.0)
                nc.gpsimd.dma_start(out=wvt_all, in_=wvt_d[:, :, :])
                nc.gpsimd.dma_start(out=wpt_all, in_=wpt_d[:, :, :])
                # stats2 cols: [mean_t0..3 | ex2_t0..3]
                stats2 = stat_sb.tile([P, 8], F32, name="stats2")
                means = mvs[:, :, 0]
                vars_ = mvs[:, :, 1]
                nc.vector.tensor_copy(stats2[:, 0:4], means)
                nc.vector.tensor_tensor(out=stats2[:, 4:8], in0=means, in1=means, op=ALU.mult)
                nc.vector.tensor_tensor(out=stats2[:, 4:8], in0=stats2[:, 4:8], in1=vars_, op=ALU.add)
                # column sums / 256 -> [1, 8] on partition 0
                ps8 = stat_ps.tile([1, 8], F32, name="ps8")
                nc.tensor.matmul(ps8, inv256, stats2, start=True, stop=True)
                s8 = stat_sb.tile([1, 8], F32, name="s8")
                nc.vector.tensor_copy(s8, ps8)
                # per-group mean and E[x^2]: adjacent-pair sums
                gme = stat_sb.tile([1, 4], F32, name="gme")  # [mu_g0, mu_g1, e_g0, e_g1]
                s8v = s8.rearrange("p (f g two) -> p f g two", f=2, two=2)
                gmev = gme.rearrange("p (f g) -> p f g", f=2)
                nc.vector.tensor_tensor(
                    out=gmev[:, :, :], in0=s8v[:, :, :, 0], in1=s8v[:, :, :, 1], op=ALU.add)
                # broadcast to 128 partitions: [128, 4]
                psb = stat_ps.tile([P, 4], F32, name="psb")
                nc.tensor.matmul(psb, ones_f32[0:1, :], gme, start=True, stop=True)
                mu_e = stat_sb.tile([P, 4], F32, name="mu_e")
                nc.vector.tensor_copy(mu_e, psb)
                mu_bc = mu_e[:, 0:2]
                e_bc = mu_e[:, 2:4]
                var_bc = stat_sb.tile([P, 2], F32, name="var_bc")
                nc.vector.tensor_tensor(out=var_bc, in0=mu_bc, in1=mu_bc, op=ALU.mult)
                nc.vector.tensor_tensor(out=var_bc, in0=e_bc, in1=var_bc, op=ALU.subtract)
                sd = stat_sb.tile([P, 2], F32, name="sd")
                for g in range(G):
                    nc.scalar.activation(out=sd[:, g:g + 1], in_=var_bc[:, g:g + 1],
                                         func=ACTF.Sqrt, bias=eps_t, scale=1.0)
                nc.scalar.activation(out=preld, in_=eps_t, func=ACTF.Exp)
                rstd = stat_sb.tile([P, 2], F32, name="rstd")
                nc.vector.reciprocal(out=rstd, in_=sd)
                # per-channel-tile affine: h = a*x + b (vectorized over tiles:
                # gnw/gnb live strided in prm as [P, CT])
                ab = persist.tile([P, 2, CT], F32, name="ab")
                a4 = ab[:, 0, :]
                b4 = ab[:, 1, :]
                gnw4 = prm[:, :, 3]
                gnb4 = prm[:, :, 4]
                for g in range(G):
                    gs = slice(2 * g, 2 * g + 2)
                    nc.vector.tensor_scalar(
                        out=a4[:, gs], in0=gnw4[:, gs], scalar1=rstd[:, g:g + 1],
                        scalar2=None, op0=ALU.mult)
                    nc.vector.tensor_scalar(
                        out=b4[:, gs], in0=a4[:, gs], scalar1=mu_bc[:, g:g + 1],
                        scalar2=None, op0=ALU.mult)
                nc.vector.tensor_tensor(out=b4, in0=gnb4, in1=b4, op=ALU.subtract)
                a_t = [a4[:, t:t + 1] for t in range(CT)]
                b_t = [b4[:, t:t + 1] for t in range(CT)]

            # ---------------- Phase 2: h + q/k/v convs (from SBUF x16) ----------------
            with (
                tc.tile_pool(name="cq_ps", bufs=2, space="PSUM") as cq_ps,
                tc.tile_pool(name="ck_ps", bufs=2, space="PSUM") as ck_ps,
                tc.tile_pool(name="cv_ps", bufs=2, space="PSUM") as cv_ps,
            ):
                for ch in range(NCH):
                    sl = slice(ch * 512, (ch + 1) * 512)
                    h16 = []
                    for t in range(CT):
                        h16t = h16_pool.tile([P, 512], F16, name="h16", tag="h16")
                        nc.vector.tensor_scalar(
                            out=h16t, in0=x16a[:, t, sl], scalar1=a_t[t], scalar2=b_t[t],
                            op0=ALU.mult, op1=ALU.add)
                        h16.append(h16t)
                    # k conv (and q for first half): single-pass fp16
                    for o in range(CT):
                        osl = slice(o * P, (o + 1) * P)
                        kp = ck_ps.tile([P, 512], F32, name="kp", tag="kp")
                        for t in range(CT):
                            nc.tensor.matmul(
                                kp, wkt[t][:, osl], h16[t],
                                start=(t == 0), stop=(t == CT - 1))
                        nc.scalar.activation(
                            out=k16[o][:, sl], in_=kp, func=ACTF.Identity,
                            bias=bk[o], scale=1.0)
                        if ch < NCH // 2:
                            qp = cq_ps.tile([P, 512], F32, name="qp", tag="qp")
                            for t in range(CT):
                                nc.tensor.matmul(
                                    qp, wqt[t][:, osl], h16[t],
                                    start=(t == 0), stop=(t == CT - 1))
                            nc.scalar.activation(
                                out=q16[o][:, sl], in_=qp, func=ACTF.Identity,
                                bias=bq[o], scale=1.0)
                    # v conv, transposed output (ch 6,7 deferred into phase 3
                    # as PE cover for block 0's softmax chains)
                    if ch < 6:
                        for mb in range(4):
                            m = ch * 4 + mb
                            vp = cv_ps.tile([P, C], F32, name="vp", tag="vp")
                            for t in range(CT):
                                nc.tensor.matmul(
                                    vp, h16[t][:, mb * P:(mb + 1) * P], wvt[t],
                                    start=(t == 0), stop=(t == CT - 1))
                            nc.vector.tensor_copy(vT[m], vp)

            # ---------------- Phase 3: attention ----------------
            with (
                tc.tile_pool(name="att_sb", bufs=1) as att_sb,
                tc.tile_pool(name="p_pool", bufs=2) as p_pool,
                tc.tile_pool(name="pt_pool", bufs=2) as pt_pool,
                tc.tile_pool(name="ot_pool", bufs=2) as ot_pool,
                tc.tile_pool(name="sc_ps", bufs=5, space="PSUM") as sc_ps,
                tc.tile_pool(name="tp_ps", bufs=1, space="PSUM") as tp_ps,
                tc.tile_pool(name="o_ps", bufs=1, space="PSUM") as o_ps,
                tc.tile_pool(name="pp_ps", bufs=1, space="PSUM") as pp_ps,
                tc.tile_pool(name="fin_pool", bufs=4) as fin_pool,
            ):
                def emit_proj_part(g2b, o, pool=None):
                    sl = slice(g2b * 256, (g2b + 1) * 256)
                    pp = (pool.tile([P, 256], F32, name="pp", tag="sp") if pool
                          else pp_ps.tile([P, 256], F32, name="pp", tag="pp"))
                    for t in range(CT):
                        nc.tensor.matmul(
                            pp, wpt[t][:, o * P:(o + 1) * P], out_c[t][:, sl],
                            start=(t == 0), stop=(t == CT - 1))
                    fin = fin_pool.tile([P, 256], F32, name="fin", tag="fin")
                    nc.scalar.activation(
                        out=fin, in_=pp, func=ACTF.Identity, bias=bp[o], scale=1.0)
                    nc.vector.tensor_tensor(
                        out=fin, in0=fin, in1=x16a[:, o, sl], op=ALU.add)
                    nc.gpsimd.dma_start(out=out_d[o][:, sl], in_=fin)

                def emit_scores_half(nb, half, st=None):
                    """One key half (4 chunks of 512), chunk-contiguous matmuls
                    with each chunk's max emitted right after its matmuls."""
                    if half == 0:
                        pt_b = p_pool.tile([P, N], F16, name="pexp", tag="pexp")
                        sums = att_sb.tile([P, 8], F32, name="sums", tag="sums", bufs=2)
                        mx = att_sb.tile([P, 8], F32, name="mx", tag="mx", bufs=2)
                        small = att_sb.tile([P, 4], F32, name="small", tag="small", bufs=2)
                    else:
                        pt_b, sums, mx, small = st
                    negm1, negm, alpha, s_tot = (small[:, i:i + 1] for i in range(4))
                    nsl = slice(nb * P, (nb + 1) * P)
                    sps = []
                    for j in range(4):
                        mch = 4 * half + j
                        msl = slice(mch * 512, (mch + 1) * 512)
                        sp = sc_ps.tile([P, 512], F32, name="sp", tag="sp")
                        for t in range(CT):
                            nc.tensor.matmul(
                                sp, q16[t][:, nsl], k16[t][:, msl],
                                start=(t == 0), stop=(t == CT - 1))
                        nc.vector.reduce_max(out=mx[:, mch:mch + 1], in_=sp, axis=AX)
                        sps.append(sp)
                    if half == 0:
                        nc.vector.reduce_max(out=negm1, in_=mx[:, 0:4], axis=AX, negate=True)
                        for j in range(4):
                            nc.scalar.activation(
                                out=pt_b[:, j * 512:(j + 1) * 512], in_=sps[j],
                                func=ACTF.Exp, bias=negm1, scale=1.0,
                                accum_out=sums[:, j:j + 1])
                        return (pt_b, sums, mx, small)
                    else:
                        nc.vector.reduce_max(out=negm, in_=mx[:, 4:8], axis=AX, negate=True)
                        nc.vector.tensor_tensor(out=negm, in0=negm, in1=negm1, op=ALU.min)
                        nc.vector.tensor_tensor(out=alpha, in0=negm, in1=negm1, op=ALU.subtract)
                        nc.scalar.activation(out=alpha, in_=alpha, func=ACTF.Exp)
                        for j in range(4):
                            nc.scalar.activation(
                                out=pt_b[:, (4 + j) * 512:(5 + j) * 512], in_=sps[j],
                                func=ACTF.Exp, bias=negm, scale=1.0,
                                accum_out=sums[:, 4 + j:5 + j])
                        return (pt_b, sums, mx, small)

                def emit_tail(st):
                    """Rescale A-half by alpha; total sum and reciprocal."""
                    pt_b, sums, mx, small = st
                    negm1, negm, alpha, s_tot = (small[:, i:i + 1] for i in range(4))
                    nc.vector.tensor_scalar_mul(
                        out=pt_b[:, 0:NQ], in0=pt_b[:, 0:NQ], scalar1=alpha)
                    nc.vector.tensor_scalar_mul(
                        out=sums[:, 0:4], in0=sums[:, 0:4], scalar1=alpha)
                    recip = att_sb.tile([P, 1], F32, name="recip", tag="recip", bufs=2)
                    nc.vector.reduce_sum(out=s_tot, in_=sums, axis=AX)
                    nc.vector.reciprocal(out=recip, in_=s_tot)
                    return recip

                def emit_v_deferred(ch):
                    """v-conv for one deferred chunk (h16 recomputed on DVE);
                    vp double-buffered through the po/pp banks."""
                    sl = slice(ch * 512, (ch + 1) * 512)
                    h16 = []
                    for t in range(CT):
                        h16t = h16_pool.tile([P, 512], F16, name="h16", tag="h16")
                        nc.vector.tensor_scalar(
                            out=h16t, in0=x16a[:, t, sl], scalar1=a_t[t], scalar2=b_t[t],
                            op0=ALU.mult, op1=ALU.add)
                        h16.append(h16t)
                    for mb in range(4):
                        m = ch * 4 + mb
                        pool, tg = (o_ps, "po") if mb % 2 else (pp_ps, "pp")
                        vp = pool.tile([P, C], F32, name="vpd", tag=tg)
                        for t in range(CT):
                            nc.tensor.matmul(
                                vp, h16[t][:, mb * P:(mb + 1) * P], wvt[t],
                                start=(t == 0), stop=(t == CT - 1))
                        nc.vector.tensor_copy(vT[m], vp)

                def emit_apply_half(nb, st, po, g2s, tpool=None):
                    """Transpose + attnV for two groups of 8 key tiles,
                    T/copy/V interleaved. Groups 2,3 (B-half of pt_b, which
                    needs no alpha rescale) run first so apply1 does not wait
                    on the previous iteration's rescale."""
                    pt_b = st[0]
                    for g2 in g2s:
                        tp = (tpool.tile([P, 1024], F16, name="tp", tag="sp")
                              if tpool else
                              tp_ps.tile([P, 1024], F16, name="tp", tag="tp"))
                        for j in range(8):
                            mt = 8 * g2 + j
                            nc.tensor.transpose(
                                tp[:, j * P:(j + 1) * P], pt_b[:, mt * P:(mt + 1) * P], ident)
                        ptg = pt_pool.tile([P, 1024], F16, name="ptg", tag="ptg")
                        nc.vector.tensor_copy(ptg, tp)
                        for j in range(8):
                            mt = 8 * g2 + j
                            nc.tensor.matmul(
                                po, ptg[:, j * P:(j + 1) * P], vT[mt],
                                start=(mt == 16), stop=(mt == 15))

                def emit_out(nb, po, recip):
                    """Normalize + transpose out_T back to [c, n]."""
                    nsl = slice(nb * P, (nb + 1) * P)
                    oT = ot_pool.tile([P, C], F16, name="oT", tag="oT")
                    nc.vector.tensor_scalar_mul(out=oT, in0=po, scalar1=recip)
                    tp2 = tp_ps.tile([P, 512], F16, name="tp2", tag="tp")
                    for t in range(CT):
                        nc.tensor.transpose(
                            tp2[:, t * P:(t + 1) * P], oT[:, t * P:(t + 1) * P], ident)
                    tp2v = tp2.rearrange("p (t n) -> p t n", t=CT)
                    nc.vector.tensor_copy(out_ca[:, :, nsl], tp2v)

                # software pipeline across iterations:
                #   iter nb: scoresA(nb) | apply1(nb-1) | scoresB(nb) | apply2(nb-1)
                #   proj for 4-block group g spread across iter 4g+5.
                prev = None         # (st, po, recip) of block nb-1
                for nb in range(NQB + 2):
                    pj = nb - 2     # proj pair-group source block
                    do_proj = pj >= 1 and (pj % 2) == 1
                    gp = (pj // 2) if do_proj else None
                    # the last proj group runs after scores are done: pipeline
                    # its psum through the freed score banks instead of pp_ps
                    pjpool = sc_ps if (do_proj and nb >= NQB + 1) else None
                    if do_proj:
                        emit_proj_part(gp, 0, pjpool)
                    stA = emit_scores_half(nb, 0) if nb < NQB else None
                    if nb == 0:
                        emit_v_deferred(6)
                    # drain phase: scores are done, pipeline the last apply's
                    # transposes through the freed score banks
                    tpool = sc_ps if nb - 1 >= NQB - 1 else None
                    if prev is not None:
                        st_p, recip_p = prev
                        po = o_ps.tile([P, C], F32, name="po", tag="po")
                        emit_apply_half(nb - 1, st_p, po, (2, 3), tpool)
                    if do_proj:
                        emit_proj_part(gp, 1, pjpool)
                    if nb < NQB:
                        stB = emit_scores_half(nb, 1, stA)
                    if nb == 0:
                        emit_v_deferred(7)
                    if do_proj:
                        emit_proj_part(gp, 2, pjpool)
                    if prev is not None:
                        emit_apply_half(nb - 1, st_p, po, (0, 1), tpool)
                        emit_out(nb - 1, po, recip_p)
                    if nb < NQB:
                        recip = emit_tail(stB)
                        prev = (stB, recip)
                    else:
                        prev = None
                    if do_proj:
                        emit_proj_part(gp, 3, pjpool)

    nc.compile()
    return nc


def _prep_shared(gn_w, gn_b, wq, bq, wk, bk, wv, bv, wp, bp):
    f32 = np.float32
    s = f32(math.sqrt(512.0))
    def pack(wT):  # [C, C] -> [P, CT, C] partition-major
        return np.ascontiguousarray(wT.reshape(CT, P, C).transpose(1, 0, 2))

    prm = np.zeros((P, CT, 6), dtype=f32)
    prm[:, :, 0] = (bq.astype(f32) * s).reshape(CT, P).T
    prm[:, :, 1] = bk.astype(f32).reshape(CT, P).T
    # v-bias folded into the proj bias: attn rows sum to 1, so
    # proj(attn@v + bv) = proj(attn@v0) + (wp@bv + bp)
    bp2 = bp.astype(f32) + wp.astype(f32) @ bv.astype(f32)
    prm[:, :, 2] = bp2.reshape(CT, P).T
    prm[:, :, 3] = gn_w.astype(f32).reshape(CT, P).T
    prm[:, :, 4] = gn_b.astype(f32).reshape(CT, P).T
    shared = {
        "wqt": pack((wq.T * s).astype(f32)).astype(np.float16),
        "wkt": pack(wk.T.astype(f32)).astype(np.float16),
        "wvt": pack(wv.T.astype(f32)).astype(np.float16),
        "wpt": pack(wp.T.astype(f32)).astype(np.float16),
        "prm": prm,
    }
    return shared


def _make_in_maps(inputs):
    x = np.asarray(inputs["x"], dtype=np.float32)
    args = [np.asarray(inputs[k], dtype=np.float32) for k in
            ("gn_w", "gn_b", "wq", "bq", "wk", "bk", "wv", "bv", "wp", "bp")]
    shared = _prep_shared(*args)
    in_maps = []
    for core in range(8):
        b, half = core // 2, core % 2
        xb = x[b].reshape(C, N)
        if half:
            xb = np.concatenate([xb[:, NQ:], xb[:, :NQ]], axis=1)
        m = dict(shared)
        m["x"] = np.ascontiguousarray(xb.reshape(CT, P, N))
        in_maps.append(m)
    return in_maps


def kernel(x, gn_w, gn_b, wq, bq, wk, bk, wv, bv, wp, bp):
    global _CACHED_NC
    if _CACHED_NC is None:
        _CACHED_NC = build_nc()
    nc = _CACHED_NC

    in_maps = _make_in_maps(dict(x=x, gn_w=gn_w, gn_b=gn_b, wq=wq, bq=bq, wk=wk,
                                 bk=bk, wv=wv, bv=bv, wp=wp, bp=bp))
    res = run_bass_kernel_spmd(nc, in_maps, core_ids=list(range(8)))

    y = np.empty((B, C, N), dtype=np.float32)
    for core in range(8):
        b, half = core // 2, core % 2
        y[b][:, half * NQ:(half + 1) * NQ] = res.results[core]["out"].reshape(C, NQ)
    return y.reshape(B, C, H, W)


# revision 12
# speedup vs baseline: 1.2266x; 1.0389x over previous
"""AttnBlock kernel for 8 Trainium2 NeuronCores — single-pass fp16.

Problem: x[4,512,64,64] f32 -> GroupNorm(2 groups, eps 1e-6) -> q,k,v 1x1
convs -> attention over N=4096 positions with scale sqrt(512) (multiplied)
-> proj -> residual.

Sharding: 8 cores = 4 examples x 2 query-halves (columns rotated per core so
its half comes first; softmax over keys is permutation invariant). No
cross-core communication.

Design:
- Phase 1: one f32 pass over x computes GroupNorm stats (bn_stats) while an
  fp16 copy of x is written to SBUF (ACT engine) for the convs and the
  residual — saves 16 MB of HBM re-reads. Sqrt/Exp ACT tables are preloaded
  in idle windows. v-bias is folded into the proj bias host-side
  (attn rows sum to 1), q weights carry the sqrt(C) scale.
- Phase 2: single-pass fp16 q/k/v convs (PE), biases applied by ACT on the
  PSUM->SBUF evacuation. v is produced transposed (positions on partitions).
  v-convs of the last 2 key chunks are deferred into phase 3 as PE cover
  for block 0's softmax chains.
- Phase 3: per 128-query block: 2-half online softmax. Score matmuls are
  chunk-contiguous so each chunk's DVE max overlaps the score stream; 5
  rotating score PSUM banks let pass B start before all of pass A is
  consumed by the ACT exps. Probabilities are PE-transposed (B-half first —
  it needs no alpha rescale), DVE-copied to SBUF, and applied against vT;
  the proj+residual of each 2-block group is spread across a following
  iteration. Apply/out for block nb runs pipelined one iteration behind.

Precision: single-pass fp16 convs/scores give rel_err ~1.03e-2 on the fixed
inputs (gate 2e-2); logits have std ~512 and the softmax is near-one-hot, so
score error is dominated by fp16 input rounding (~0.4 abs on logits), which
flips only near-degenerate argmaxes. GN stats and softmax run in fp32.
"""

import math

import numpy as np

import concourse.bacc as bacc
import concourse.mybir as mybir
import concourse.tile as tile
from concourse.bass_utils import run_bass_kernel_spmd
from concourse.masks import make_identity

F32 = mybir.dt.float32
F16 = mybir.dt.float16

B, C, H, W = 4, 512, 64, 64
N = H * W            # 4096 key positions
NQ = N // 2          # 2048 query positions per core
P = 128              # partitions
CT = C // P          # 4 channel tiles
NCH = N // 512       # 8 key chunks of 512
NQB = NQ // P        # 16 query blocks of 128
G = 2                # groupnorm groups
EPS = 1e-6
AX = mybir.AxisListType.X
ALU = mybir.AluOpType
ACTF = mybir.ActivationFunctionType

_CACHED_NC = None


def build_nc(loop_r: int = 1):
    nc = bacc.Bacc("TRN2", target_bir_lowering=False)

    x_d = nc.dram_tensor("x", [CT, P, N], F32, kind="ExternalInput")
    wqt_d = nc.dram_tensor("wqt", [P, CT, C], F16, kind="ExternalInput")  # [p, t, o], q scaled by sqrt(C)
    wkt_d = nc.dram_tensor("wkt", [P, CT, C], F16, kind="ExternalInput")
    wvt_d = nc.dram_tensor("wvt", [P, CT, C], F16, kind="ExternalInput")
    wpt_d = nc.dram_tensor("wpt", [P, CT, C], F16, kind="ExternalInput")
    # per-channel params packed: [p, t, (bq, bk, bp', gnw, gnb, pad)]
    # bp' = bp + wp @ bv  (v-bias folded into proj bias; attn rows sum to 1)
    prm_d = nc.dram_tensor("prm", [P, CT, 6], F32, kind="ExternalInput")
    out_d = nc.dram_tensor("out", [CT, P, NQ], F32, kind="ExternalOutput")

    import contextlib

    with tile.TileContext(nc) as tc:
        loop_ctx = tc.For_i(0, loop_r, 1) if loop_r > 1 else contextlib.nullcontext()
        with (
            loop_ctx,
            tc.tile_pool(name="singles", bufs=1) as singles,
            tc.tile_pool(name="persist", bufs=1) as persist,
            tc.tile_pool(name="h16_pool", bufs=8) as h16_pool,
        ):
            ident = singles.tile([P, P], F16, name="ident")
            make_identity(nc, ident)
            ones_f32 = singles.tile([P, P], F32, name="ones_f32")
            nc.vector.memset(ones_f32, 1.0)
            inv256 = singles.tile([P, 1], F32, name="inv256")
            nc.vector.memset(inv256, 1.0 / 256.0)
            eps_t = singles.tile([P, 1], F32, name="eps_t")
            nc.vector.memset(eps_t, EPS)

            # weights and per-channel params: one packed DMA each
            wqt_all = persist.tile([P, CT, C], F16, name="wqt_all")
            wkt_all = persist.tile([P, CT, C], F16, name="wkt_all")
            wvt_all = persist.tile([P, CT, C], F16, name="wvt_all")
            wpt_all = persist.tile([P, CT, C], F16, name="wpt_all")
            prm = persist.tile([P, CT, 6], F32, name="prm")
            # prm loads early (needed for the stats tail); all weights queue on
            # the sync queue behind the phase-1 x stream so they don't steal
            # HBM bandwidth from it — they arrive just in time for phase 2
            nc.gpsimd.dma_start(out=prm, in_=prm_d[:, :, :])
            nc.gpsimd.dma_start(out=wkt_all, in_=wkt_d[:, :, :])
            nc.gpsimd.dma_start(out=wqt_all, in_=wqt_d[:, :, :])
            wqt = [wqt_all[:, t, :] for t in range(CT)]
            wkt = [wkt_all[:, t, :] for t in range(CT)]
            wvt = [wvt_all[:, t, :] for t in range(CT)]
            wpt = [wpt_all[:, t, :] for t in range(CT)]
            bq = [prm[:, t, 0:1] for t in range(CT)]
            bk = [prm[:, t, 1:2] for t in range(CT)]
            bp = [prm[:, t, 2:3] for t in range(CT)]
            gnw = [prm[:, t, 3:4] for t in range(CT)]
            gnb = [prm[:, t, 4:5] for t in range(CT)]

            # persistent activations (single-pass fp16)
            x16a = persist.tile([P, CT, N], F16, name="x16a")
            k16 = [persist.tile([P, N], F16, name=f"k16_{t}") for t in range(CT)]
            q16 = [persist.tile([P, NQ], F16, name=f"q16_{t}") for t in range(CT)]
            vTa = persist.tile([P, N // P, C], F16, name="vTa")
            vT = [vTa[:, m, :] for m in range(N // P)]
            out_ca = persist.tile([P, CT, NQ], F16, name="out_ca")
            out_c = [out_ca[:, t, :] for t in range(CT)]

            # ---------------- Phase 1: GroupNorm statistics + x16 ----------------
            with (
                tc.tile_pool(name="stat_sb", bufs=1) as stat_sb,
                tc.tile_pool(name="stat_ps", bufs=2, space="PSUM") as stat_ps,
            ):
                stats6 = [stat_sb.tile([P, NCH, 6], F32, name=f"st6_{t}") for t in range(CT)]
                mvs = stat_sb.tile([P, CT, 2], F32, name="mvs")
                for t in range(CT):
                    for hf in range(2):
                        sl = slice(hf * (N // 2), (hf + 1) * (N // 2))
                        xb = stat_sb.tile([P, N // 2], F32, name="xbig", tag="xbig", bufs=3)
                        nc.sync.dma_start(out=xb, in_=x_d[t][:, sl])
                        for c2 in range(NCH // 2):
                            ch = hf * (NCH // 2) + c2
                            nc.vector.bn_stats(
                                out=stats6[t][:, ch, :], in_=xb[:, c2 * 512:(c2 + 1) * 512])
                        nc.scalar.activation(
                            out=x16a[:, t, sl], in_=xb, func=ACTF.Identity, scale=1.0)
                    nc.vector.bn_aggr(out=mvs[:, t, :], in_=stats6[t])
                # preload the Sqrt table set while ACT is idle (a lazy load
                # would land on the stats critical path)
                preld = stat_sb.tile([P, 1], F32, name="preld")
                nc.scalar.activation(out=preld, in_=eps_t, func=ACTF.Sqrt,
                                     bias=eps_t, scale=1.0)
                nc.gpsimd.dma_start(out=wvt_all, in_=wvt_d[:, :, :])
                nc.gpsimd.dma_start(out=wpt_all, in_=wpt_d[:, :, :])
                # stats2 cols: [mean_t0..3 | ex2_t0..3]
                stats2 = stat_sb.tile([P, 8], F32, name="stats2")
                means = mvs[:, :, 0]
                vars_ = mvs[:, :, 1]
                nc.vector.tensor_copy(stats2[:, 0:4], means)
                nc.vector.tensor_tensor(out=stats2[:, 4:8], in0=means, in1=means, op=ALU.mult)
                nc.vector.tensor_tensor(out=stats2[:, 4:8], in0=stats2[:, 4:8], in1=vars_, op=ALU.add)
                # column sums / 256 -> [1, 8] on partition 0
                ps8 = stat_ps.tile([1, 8], F32, name="ps8")
                nc.tensor.matmul(ps8, inv256, stats2, start=True, stop=True)
                s8 = stat_sb.tile([1, 8], F32, name="s8")
                nc.vector.tensor_copy(s8, ps8)
                # per-group mean and E[x^2]: adjacent-pair sums
                gme = stat_sb.tile([1, 4], F32, name="gme")  # [mu_g0, mu_g1, e_g0, e_g1]
                s8v = s8.rearrange("p (f g two) -> p f g two", f=2, two=2)
                gmev = gme.rearrange("p (f g) -> p f g", f=2)
                nc.vector.tensor_tensor(
                    out=gmev[:, :, :], in0=s8v[:, :, :, 0], in1=s8v[:, :, :, 1], op=ALU.add)
                # broadcast to 128 partitions: [128, 4]
                psb = stat_ps.tile([P, 4], F32, name="psb")
                nc.tensor.matmul(psb, ones_f32[0:1, :], gme, start=True, stop=True)
                mu_e = stat_sb.tile([P, 4], F32, name="mu_e")
                nc.vector.tensor_copy(mu_e, psb)
                mu_bc = mu_e[:, 0:2]
                e_bc = mu_e[:, 2:4]
                var_bc = stat_sb.tile([P, 2], F32, name="var_bc")
                nc.vector.tensor_tensor(out=var_bc, in0=mu_bc, in1=mu_bc, op=ALU.mult)
                nc.vector.tensor_tensor(out=var_bc, in0=e_bc, in1=var_bc, op=ALU.subtract)
                sd = stat_sb.tile([P, 2], F32, name="sd")
                for g in range(G):
                    nc.scalar.activation(out=sd[:, g:g + 1], in_=var_bc[:, g:g + 1],
                                         func=ACTF.Sqrt, bias=eps_t, scale=1.0)
                nc.scalar.activation(out=preld, in_=eps_t, func=ACTF.Exp)
                rstd = stat_sb.tile([P, 2], F32, name="rstd")
                nc.vector.reciprocal(out=rstd, in_=sd)
                # per-channel-tile affine: h = a*x + b (vectorized over tiles:
                # gnw/gnb live strided in prm as [P, CT])
                ab = persist.tile([P, 2, CT], F32, name="ab")
                a4 = ab[:, 0, :]
                b4 = ab[:, 1, :]
                gnw4 = prm[:, :, 3]
                gnb4 = prm[:, :, 4]
                for g in range(G):
                    gs = slice(2 * g, 2 * g + 2)
                    nc.vector.tensor_scalar(
                        out=a4[:, gs], in0=gnw4[:, gs], scalar1=rstd[:, g:g + 1],
                        scalar2=None, op0=ALU.mult)
                    nc.vector.tensor_scalar(
                        out=b4[:, gs], in0=a4[:, gs], scalar1=mu_bc[:, g:g + 1],
                        scalar2=None, op0=ALU.mult)
                nc.vector.tensor_tensor(out=b4, in0=gnb4, in1=b4, op=ALU.subtract)
                a_t = [a4[:, t:t + 1] for t in range(CT)]
                b_t = [b4[:, t:t + 1] for t in range(CT)]

            # ---------------- Phase 2: h + q/k/v convs (from SBUF x16) ----------------
            with (
                tc.tile_pool(name="cq_ps", bufs=2, space="PSUM") as cq_ps,
                tc.tile_pool(name="ck_ps", bufs=2, space="PSUM") as ck_ps,
                tc.tile_pool(name="cv_ps", bufs=2, space="PSUM") as cv_ps,
            ):
                for ch in range(NCH):
                    sl = slice(ch * 512, (ch + 1) * 512)
                    h16 = []
                    for t in range(CT):
                        h16t = h16_pool.tile([P, 512], F16, name="h16", tag="h16")
                        nc.vector.tensor_scalar(
                            out=h16t, in0=x16a[:, t, sl], scalar1=a_t[t], scalar2=b_t[t],
                            op0=ALU.mult, op1=ALU.add)
                        h16.append(h16t)
                    # k conv (and q for first half): single-pass fp16
                    for o in range(CT):
                        osl = slice(o * P, (o + 1) * P)
                        kp = ck_ps.tile([P, 512], F32, name="kp", tag="kp")
                        for t in range(CT):
                            nc.tensor.matmul(
                                kp, wkt[t][:, osl], h16[t],
                                start=(t == 0), stop=(t == CT - 1))
                        nc.scalar.activation(
                            out=k16[o][:, sl], in_=kp, func=ACTF.Identity,
                            bias=bk[o], scale=1.0)
                        if ch < NCH // 2:
                            qp = cq_ps.tile([P, 512], F32, name="qp", tag="qp")
                            for t in range(CT):
                                nc.tensor.matmul(
                                    qp, wqt[t][:, osl], h16[t],
                                    start=(t == 0), stop=(t == CT - 1))
                            nc.scalar.activation(
                                out=q16[o][:, sl], in_=qp, func=ACTF.Identity,
                                bias=bq[o], scale=1.0)
                    # v conv, transposed output (ch 6,7 deferred into phase 3
                    # as PE cover for block 0's softmax chains)
                    if ch < 6:
                        for mb in range(4):
                            m = ch * 4 + mb
                            vp = cv_ps.tile([P, C], F32, name="vp", tag="vp")
                            for t in range(CT):
                                nc.tensor.matmul(
                                    vp, h16[t][:, mb * P:(mb + 1) * P], wvt[t],
                                    start=(t == 0), stop=(t == CT - 1))
                            nc.vector.tensor_copy(vT[m], vp)

            # ---------------- Phase 3: attention ----------------
            with (
                tc.tile_pool(name="att_sb", bufs=1) as att_sb,
                tc.tile_pool(name="p_pool", bufs=2) as p_pool,
                tc.tile_pool(name="pt_pool", bufs=2) as pt_pool,
                tc.tile_pool(name="ot_pool", bufs=2) as ot_pool,
                tc.tile_pool(name="sc_ps", bufs=5, space="PSUM") as sc_ps,
                tc.tile_pool(name="tp_ps", bufs=1, space="PSUM") as tp_ps,
                tc.tile_pool(name="o_ps", bufs=1, space="PSUM") as o_ps,
                tc.tile_pool(name="pp_ps", bufs=1, space="PSUM") as pp_ps,
                tc.tile_pool(name="fin_pool", bufs=4) as fin_pool,
            ):
                def emit_proj_part(g2b, o, pool=None):
                    sl = slice(g2b * 256, (g2b + 1) * 256)
                    pp = (pool.tile([P, 256], F32, name="pp", tag="sp") if pool
                          else pp_ps.tile([P, 256], F32, name="pp", tag="pp"))
                    for t in range(CT):
                        nc.tensor.matmul(
                            pp, wpt[t][:, o * P:(o + 1) * P], out_c[t][:, sl],
                            start=(t == 0), stop=(t == CT - 1))
                    fin = fin_pool.tile([P, 256], F32, name="fin", tag="fin")
                    nc.scalar.activation(
                        out=fin, in_=pp, func=ACTF.Identity, bias=bp[o], scale=1.0)
                    nc.vector.tensor_tensor(
                        out=fin, in0=fin, in1=x16a[:, o, sl], op=ALU.add)
                    nc.gpsimd.dma_start(out=out_d[o][:, sl], in_=fin)

                def emit_scores_half(nb, half, st=None):
                    """One key half (4 chunks of 512), chunk-contiguous matmuls
                    with each chunk's max emitted right after its matmuls."""
                    if half == 0:
                        pt_b = p_pool.tile([P, N], F16, name="pexp", tag="pexp")
                        sums = att_sb.tile([P, 8], F32, name="sums", tag="sums", bufs=2)
                        mx = att_sb.tile([P, 8], F32, name="mx", tag="mx", bufs=2)
                        small = att_sb.tile([P, 4], F32, name="small", tag="small", bufs=2)
                    else:
                        pt_b, sums, mx, small = st
                    negm1, negm, alpha, s_tot = (small[:, i:i + 1] for i in range(4))
                    nsl = slice(nb * P, (nb + 1) * P)
                    sps = []
                    for j in range(4):
                        mch = 4 * half + j
                        msl = slice(mch * 512, (mch + 1) * 512)
                        sp = sc_ps.tile([P, 512], F32, name="sp", tag="sp")
                        for t in range(CT):
                            nc.tensor.matmul(
                                sp, q16[t][:, nsl], k16[t][:, msl],
                                start=(t == 0), stop=(t == CT - 1))
                        nc.vector.reduce_max(out=mx[:, mch:mch + 1], in_=sp, axis=AX)
                        sps.append(sp)
                    if half == 0:
                        nc.vector.reduce_max(out=negm1, in_=mx[:, 0:4], axis=AX, negate=True)
                        for j in range(4):
                            nc.scalar.activation(
                                out=pt_b[:, j * 512:(j + 1) * 512], in_=sps[j],
                                func=ACTF.Exp, bias=negm1, scale=1.0,
                                accum_out=sums[:, j:j + 1])
                        return (pt_b, sums, mx, small)
                    else:
                        nc.vector.reduce_max(out=negm, in_=mx[:, 4:8], axis=AX, negate=True)
                        nc.vector.tensor_tensor(out=negm, in0=negm, in1=negm1, op=ALU.min)
                        nc.vector.tensor_tensor(out=alpha, in0=negm, in1=negm1, op=ALU.subtract)
                        nc.scalar.activation(out=alpha, in_=alpha, func=ACTF.Exp)
                        for j in range(4):
                            nc.scalar.activation(
                                out=pt_b[:, (4 + j) * 512:(5 + j) * 512], in_=sps[j],
                                func=ACTF.Exp, bias=negm, scale=1.0,
                                accum_out=sums[:, 4 + j:5 + j])
                        return (pt_b, sums, mx, small)

                def emit_tail(st):
                    """Rescale A-half by alpha; total sum and reciprocal."""
                    pt_b, sums, mx, small = st
                    negm1, negm, alpha, s_tot = (small[:, i:i + 1] for i in range(4))
                    nc.vector.tensor_scalar_mul(
                        out=pt_b[:, 0:NQ], in0=pt_b[:, 0:NQ], scalar1=alpha)
                    nc.vector.tensor_scalar_mul(
                        out=sums[:, 0:4], in0=sums[:, 0:4], scalar1=alpha)
                    recip = att_sb.tile([P, 1], F32, name="recip", tag="recip", bufs=2)
                    nc.vector.reduce_sum(out=s_tot, in_=sums, axis=AX)
                    nc.vector.reciprocal(out=recip, in_=s_tot)
                    return recip

                def emit_v_deferred(ch):
                    """v-conv for one deferred chunk (h16 recomputed on DVE);
                    vp double-buffered through the po/pp banks."""
                    sl = slice(ch * 512, (ch + 1) * 512)
                    h16 = []
                    for t in range(CT):
                        h16t = h16_pool.tile([P, 512], F16, name="h16", tag="h16")
                        nc.vector.tensor_scalar(
                            out=h16t, in0=x16a[:, t, sl], scalar1=a_t[t], scalar2=b_t[t],
                            op0=ALU.mult, op1=ALU.add)
                        h16.append(h16t)
                    for mb in range(4):
                        m = ch * 4 + mb
                        pool, tg = (o_ps, "po") if mb % 2 else (pp_ps, "pp")
                        vp = pool.tile([P, C], F32, name="vpd", tag=tg)
                        for t in range(CT):
                            nc.tensor.matmul(
                                vp, h16[t][:, mb * P:(mb + 1) * P], wvt[t],
                                start=(t == 0), stop=(t == CT - 1))
                        nc.vector.tensor_copy(vT[m], vp)

                def emit_apply_half(nb, st, po, g2s, tpool=None):
                    """Transpose + attnV for two groups of 8 key tiles,
                    T/copy/V interleaved. Groups 2,3 (B-half of pt_b, which
                    needs no alpha rescale) run first so apply1 does not wait
                    on the previous iteration's rescale."""
                    pt_b = st[0]
                    for g2 in g2s:
                        tp = (tpool.tile([P, 1024], F16, name="tp", tag="sp")
                              if tpool else
                              tp_ps.tile([P, 1024], F16, name="tp", tag="tp"))
                        for j in range(8):
                            mt = 8 * g2 + j
                            nc.tensor.transpose(
                                tp[:, j * P:(j + 1) * P], pt_b[:, mt * P:(mt + 1) * P], ident)
                        ptg = pt_pool.tile([P, 1024], F16, name="ptg", tag="ptg")
                        nc.vector.tensor_copy(ptg, tp)
                        for j in range(8):
                            mt = 8 * g2 + j
                            nc.tensor.matmul(
                                po, ptg[:, j * P:(j + 1) * P], vT[mt],
                                start=(mt == 16), stop=(mt == 15))

                def emit_out(nb, po, recip):
                    """Normalize + transpose out_T back to [c, n]."""
                    nsl = slice(nb * P, (nb + 1) * P)
                    oT = ot_pool.tile([P, C], F16, name="oT", tag="oT")
                    nc.vector.tensor_scalar_mul(out=oT, in0=po, scalar1=recip)
                    tp2 = tp_ps.tile([P, 512], F16, name="tp2", tag="tp")
                    for t in range(CT):
                        nc.tensor.transpose(
                            tp2[:, t * P:(t + 1) * P], oT[:, t * P:(t + 1) * P], ident)
                    tp2v = tp2.rearrange("p (t n) -> p t n", t=CT)
                    nc.vector.tensor_copy(out_ca[:, :, nsl], tp2v)

                # software pipeline across iterations:
                #   iter nb: scoresA(nb) | apply1(nb-1) | scoresB(nb) | apply2(nb-1)
                #   proj for 4-block group g spread across iter 4g+5.
                prev = None         # (st, po, recip) of block nb-1
                for nb in range(NQB + 2):
                    pj = nb - 2     # proj pair-group source block
                    do_proj = pj >= 1 and (pj % 2) == 1
                    gp = (pj // 2) if do_proj else None
                    # the last proj group runs after scores are done: pipeline
                    # its psum through the freed score banks instead of pp_ps
                    pjpool = sc_ps if (do_proj and nb >= NQB + 1) else None
                    if do_proj:
                        emit_proj_part(gp, 0, pjpool)
                    stA = emit_scores_half(nb, 0) if nb < NQB else None
                    if nb == 0:
                        emit_v_deferred(6)
                    # drain phase: scores are done, pipeline the last apply's
                    # transposes through the freed score banks
                    tpool = sc_ps if nb - 1 >= NQB - 1 else None
                    if prev is not None:
                        st_p, recip_p = prev
                        po = o_ps.tile([P, C], F32, name="po", tag="po")
                        emit_apply_half(nb - 1, st_p, po, (2, 3), tpool)
                    if do_proj:
                        emit_proj_part(gp, 1, pjpool)
                    if nb < NQB:
                        stB = emit_scores_half(nb, 1, stA)
                    if nb == 0:
                        emit_v_deferred(7)
                    if do_proj:
                        emit_proj_part(gp, 2, pjpool)
                    if prev is not None:
                        emit_apply_half(nb - 1, st_p, po, (0, 1), tpool)
                        emit_out(nb - 1, po, recip_p)
                    if nb < NQB:
                        recip = emit_tail(stB)
                        prev = (stB, recip)
                    else:
                        prev = None
                    if do_proj:
                        emit_proj_part(gp, 3, pjpool)

    nc.compile()
    return nc


def _prep_shared(gn_w, gn_b, wq, bq, wk, bk, wv, bv, wp, bp):
    f32 = np.float32
    s = f32(math.sqrt(512.0))
    def pack(wT):  # [C, C] -> [P, CT, C] partition-major
        return np.ascontiguousarray(wT.reshape(CT, P, C).transpose(1, 0, 2))

    prm = np.zeros((P, CT, 6), dtype=f32)
    prm[:, :, 0] = (bq.astype(f32) * s).reshape(CT, P).T
    prm[:, :, 1] = bk.astype(f32).reshape(CT, P).T
    # v-bias folded into the proj bias: attn rows sum to 1, so
    # proj(attn@v + bv) = proj(attn@v0) + (wp@bv + bp)
    bp2 = bp.astype(f32) + wp.astype(f32) @ bv.astype(f32)
    prm[:, :, 2] = bp2.reshape(CT, P).T
    prm[:, :, 3] = gn_w.astype(f32).reshape(CT, P).T
    prm[:, :, 4] = gn_b.astype(f32).reshape(CT, P).T
    shared = {
        "wqt": pack((wq.T * s).astype(f32)).astype(np.float16),
        "wkt": pack(wk.T.astype(f32)).astype(np.float16),
        "wvt": pack(wv.T.astype(f32)).astype(np.float16),
        "wpt": pack(wp.T.astype(f32)).astype(np.float16),
        "prm": prm,
    }
    return shared


def _make_in_maps(inputs):
    x = np.asarray(inputs["x"], dtype=np.float32)
    args = [np.asarray(inputs[k], dtype=np.float32) for k in
            ("gn_w", "gn_b", "wq", "bq", "wk", "bk", "wv", "bv", "wp", "bp")]
    shared = _prep_shared(*args)
    in_maps = []
    for core in range(8):
        b, half = core // 2, core % 2
        xb = x[b].reshape(C, N)
        if half:
            xb = np.concatenate([xb[:, NQ:], xb[:, :NQ]], axis=1)
        m = dict(shared)
        m["x"] = np.ascontiguousarray(xb.reshape(CT, P, N))
        in_maps.append(m)
    return in_maps


def kernel(x, gn_w, gn_b, wq, bq, wk, bk, wv, bv, wp, bp):
    global _CACHED_NC
    if _CACHED_NC is None:
        _CACHED_NC = build_nc()
    nc = _CACHED_NC

    in_maps = _make_in_maps(dict(x=x, gn_w=gn_w, gn_b=gn_b, wq=wq, bq=bq, wk=wk,
                                 bk=bk, wv=wv, bv=bv, wp=wp, bp=bp))
    res = run_bass_kernel_spmd(nc, in_maps, core_ids=list(range(8)))

    y = np.empty((B, C, N), dtype=np.float32)
    for core in range(8):
        b, half = core // 2, core % 2
        y[b][:, half * NQ:(half + 1) * NQ] = res.results[core]["out"].reshape(C, NQ)
    return y.reshape(B, C, H, W)
